# revision 1
# baseline (speedup 1.0000x reference)
"""Trainium2 Bass kernel for LocalCrossCorrelationWithSmoothnessLoss.

Full inputs in, full output out. Internally: pure data-parallel over the
batch dim (B=8 -> 8 NeuronCores); each core computes partial sums for its
image; the host combines them into the three scalar losses.

Per-core pipeline (one 1024x1024 image pair + two flow channels):
  products   IJ = I*J (DVE), I2 = I^2, J2 = J^2 (ACT), bf16
  stage 1    H-direction 9-tap box conv as banded matmuls on the PE
             (band stationary, map moving, bf16, fp32 accumulate).
             Product maps use an 81-scaled band so the later combine is
             pure tensor-tensor work (81*S_IJ - S_I*S_J etc.).
  transpose  PE transpose per 120-row chunk (chunk-aligned, w-halo baked
             into the source free-dim offsets)
  stage 2    W-direction box conv, same banded matmuls on transposed maps
  combine    crossN = 81S_IJ - S_I*S_J, IvarN = 81S_II - S_I^2,
             JvarN = 81S_JJ - S_J^2   (all plain TT)
             cc = crossN^2 * exp(-ln(IvarN*JvarN))   (ln/exp on ACT, fp32)
             accumulated per-partition via STT accum_out
  smooth     sum(s^2) (ACT accum), lag products sum(s[w]s[w+1]) and
             sum(s[h]s[h+1]) (STT accum; row shift via SBUF->SBUF DMA).
             Edge-column/row corrections are computed on the host.

Output per core: 8 partial sums. Host assembles the losses in float64.
"""
import sys
import types
import numpy as np

sys.path.insert(0, "/opt/trn_rl_repo")

import ml_dtypes
import bass_rust
import concourse.bass as bass
import concourse.tile as tile
from concourse import mybir
from concourse import bass_utils
from concourse import tile_utils

F32 = mybir.dt.float32
F32R = mybir.dt.float32r
BF16 = mybir.dt.bfloat16
ALU = mybir.AluOpType
ACTF = mybir.ActivationFunctionType

H = 1024
W = 1024
PAD = 4
WIN = 81.0
ALPHA = 0.01
EPS = 1e-9
STRIDE = 120

# chunk table: (out_lo, out_n, in_lo, in_n)
CHUNKS = []
for _c in range((H + STRIDE - 1) // STRIDE):
    _olo = STRIDE * _c
    _on = min(STRIDE, H - _olo)
    _ilo = max(0, _olo - PAD)
    _ihi = min(H, _olo + _on + PAD)
    CHUNKS.append((_olo, _on, _ilo, _ihi - _ilo))
NCH = len(CHUNKS)

# allow using the full usable SBUF (tile_utils default is stale at 192K)
tile_utils.max_sbuf_usage = 206 * 1024

_nc_cache = {}


def _legalize_waits(nc, max_waits=1):
    """walrus here accepts only one sync-wait command per instruction;
    split extras onto same-engine NoOps placed just before."""
    ctr = 0
    for f in nc.m.functions:
        for bb in f.blocks:
            insts = bb.instructions
            i = 0
            while i < len(insts):
                ins = insts[i]
                si = ins.sync_info
                if si is None:
                    i += 1
                    continue
                w = list(si.on_wait)
                if len(w) <= max_waits:
                    i += 1
                    continue
                extra, keep = w[:-max_waits], w[-max_waits:]
                nops = []
                for j in range(0, len(extra), max_waits):
                    chunk = extra[j:j + max_waits]
                    nop = mybir.InstNoOp(name=f"I-wsplit-{ctr}", ins=[], outs=[])
                    ctr += 1
                    nop.engine = ins.engine
                    nop.sync_info = bass_rust.SyncInfo(on_wait=chunk, on_update=[])
                    nops.append(nop)
                ins.sync_info = bass_rust.SyncInfo(on_wait=keep,
                                                  on_update=list(si.on_update))
                insts[i:i] = nops
                i += len(nops) + 1


def _make_host_consts():
    """Band matrices (bf16), identity (bf16), ones (f32)."""
    def band(klo, kn, olo, on, scale):
        k = np.arange(klo, klo + kn)[:, None]
        m = np.arange(olo, olo + on)[None, :]
        return (np.abs(k - m) <= PAD).astype(np.float32) * scale

    bands = np.zeros((128, 4 * STRIDE), dtype=np.float32)
    # variant 0: first chunk (c=0), scale 1;  variant 1: first chunk, 81
    # variant 2: interior (c>=1), scale 1;    variant 3: interior, 81
    olo0, on0, ilo0, in0 = CHUNKS[0]
    bands[:in0, 0:on0] = band(ilo0, in0, olo0, on0, 1.0)
    bands[:in0, STRIDE:STRIDE + on0] = band(ilo0, in0, olo0, on0, 81.0)
    olo1, on1, ilo1, in1 = CHUNKS[1]
    bands[:in1, 2 * STRIDE:2 * STRIDE + on1] = band(ilo1, in1, olo1, on1, 1.0)
    bands[:in1, 3 * STRIDE:3 * STRIDE + on1] = band(ilo1, in1, olo1, on1, 81.0)
    bands_bf = bands.astype(ml_dtypes.bfloat16)
    ident_bf = np.eye(128, dtype=np.float32).astype(ml_dtypes.bfloat16)
    ones_f32 = np.ones((128, 1), dtype=np.float32)
    return bands_bf, bands, ident_bf, ones_f32


def _band_ap(bands_t, c, scaled):
    """AP into the packed bands tile for chunk c."""
    olo, on, ilo, inn = CHUNKS[c]
    if c == 0:
        v = 1 if scaled else 0
    else:
        v = 3 if scaled else 2
    return bands_t[0:inn, v * STRIDE:v * STRIDE + on]


def _build(nc):
    I_d = nc.dram_tensor("I", [H, W], F32, kind="ExternalInput").ap()
    J_d = nc.dram_tensor("J", [H, W], F32, kind="ExternalInput").ap()
    s0_d = nc.dram_tensor("s0", [H, W], F32, kind="ExternalInput").ap()
    s1_d = nc.dram_tensor("s1", [H, W], F32, kind="ExternalInput").ap()
    bands_d = nc.dram_tensor("bands", [128, 4 * STRIDE], BF16,
                             kind="ExternalInput").ap()
    bandsr_d = nc.dram_tensor("bandsr", [128, 4 * STRIDE], F32R,
                              kind="ExternalInput").ap()
    ident_d = nc.dram_tensor("ident", [128, 128], BF16,
                             kind="ExternalInput").ap()
    ones_d = nc.dram_tensor("ones", [128, 1], F32, kind="ExternalInput").ap()
    part_d = nc.dram_tensor("partials", [1, 68], F32,
                            kind="ExternalOutput").ap()

    from contextlib import ExitStack
    with tile.TileContext(nc) as tc, ExitStack() as ctx:
        consts = ctx.enter_context(tc.tile_pool(name="consts", bufs=1))
        inp = ctx.enter_context(tc.tile_pool(name="inp", bufs=2))
        prod = ctx.enter_context(tc.tile_pool(name="prod", bufs=2))
        smap = ctx.enter_context(tc.tile_pool(name="smap", bufs=1))
        tmap = ctx.enter_context(tc.tile_pool(name="tmap", bufs=2))
        ctmp = ctx.enter_context(tc.tile_pool(name="ctmp", bufs=2))
        spool = ctx.enter_context(tc.tile_pool(name="spool", bufs=2))
        accp = ctx.enter_context(tc.tile_pool(name="accp", bufs=1))
        psA = ctx.enter_context(tc.tile_pool(name="psA", bufs=2, space="PSUM"))
        ps2 = ctx.enter_context(tc.tile_pool(name="ps2", bufs=1, space="PSUM"))
        psF = ctx.enter_context(tc.tile_pool(name="psF", bufs=1, space="PSUM"))

        bands_t = consts.tile([128, 4 * STRIDE], BF16)
        bandsr_t = consts.tile([128, 4 * STRIDE], F32R)
        ident_t = consts.tile([128, 128], BF16)
        ones_t = consts.tile([128, 1], F32)
        nc.sync.dma_start(bands_t[:], bands_d)
        nc.sync.dma_start(bandsr_t[:], bandsr_d)
        nc.sync.dma_start(ident_t[:], ident_d)
        nc.sync.dma_start(ones_t[:], ones_d)

        # accumulators: accum_out OVERWRITES, so every accumulating
        # instruction gets its own column; host sums the groups.
        # cols 0-17: cc per (chunk,half); 18-33: lag_w; 34-49: lag_h;
        # 50-51: lag_h boundary; 52-67: s^2
        acc = accp.tile([128, 68], F32)
        nc.vector.memset(acc[:], 0.0)

        # ---------------- stage 1: H-conv -> S maps --------------------
        # S maps: per map 9 chunk tiles [out_n<=120, W] bf16, persistent
        MAPS = ("si", "sj", "sij", "sii", "sjj")
        s_tiles = {}
        for c, (olo, on, ilo, inn) in enumerate(CHUNKS):
            I_t = inp.tile([128, W], F32, tag="I_in")
            J_t = inp.tile([128, W], F32, tag="J_in")
            nc.sync.dma_start(I_t[0:inn, :], I_d[ilo:ilo + inn, :])
            nc.scalar.dma_start(J_t[0:inn, :], J_d[ilo:ilo + inn, :])

            sts = {}
            for name in MAPS:
                sts[name] = smap.tile([128, W], BF16, tag=f"S_{name}_{c}",
                                      name=f"S_{name}_{c}")
                s_tiles[(name, c)] = sts[name]
            for hw in range(2):
                wsl = slice(512 * hw, 512 * hw + 512)
                I_r = prod.tile([128, 512], F32R, tag="I_r")
                J_r = prod.tile([128, 512], F32R, tag="J_r")
                nc.vector.tensor_copy(I_r[0:inn, :], I_t[0:inn, wsl])
                nc.vector.tensor_copy(J_r[0:inn, :], J_t[0:inn, wsl])
                IJ_r = prod.tile([128, 512], F32R, tag="IJ_r")
                nc.vector.tensor_tensor(out=IJ_r[0:inn, :],
                                        in0=I_t[0:inn, wsl],
                                        in1=J_t[0:inn, wsl], op=ALU.mult)
                I2_r = prod.tile([128, 512], F32R, tag="I2_r")
                J2_r = prod.tile([128, 512], F32R, tag="J2_r")
                nc.scalar.square(I2_r[0:inn, :], I_t[0:inn, wsl])
                nc.scalar.square(J2_r[0:inn, :], J_t[0:inn, wsl])
                srcs = (I_r, J_r, IJ_r, I2_r, J2_r)
                for mi, name in enumerate(MAPS):
                    scaled = mi >= 2
                    p1 = psA.tile([128, 512], F32, tag="psA",
                                  padded_shape=[128, 512])
                    nc.tensor.matmul(p1[0:on, :],
                                     _band_ap(bandsr_t, c, scaled),
                                     srcs[mi][0:inn, :],
                                     start=True, stop=True)
                    if (c * 10 + mi * 2 + hw) % 2 == 0:
                        nc.vector.tensor_copy(sts[name][0:on, wsl],
                                              p1[0:on, :])
                    else:
                        nc.scalar.copy(sts[name][0:on, wsl], p1[0:on, :])

        # ------------- stage 2 per chunk: transpose, W-conv, combine ----
        for c2, (olo2, on2, ilo2, in2) in enumerate(CHUNKS):
            n = on2
            t_tiles = {}
            for mi, name in enumerate(MAPS):
                # transpose all 9 h'-segments into one bf16 psum bank
                pT = psA.tile([128, H], BF16, tag="psA", name="pT")
                for ch, (holo, hon, _, _) in enumerate(CHUNKS):
                    st = s_tiles[(name, ch)]
                    nc.tensor.matmul(
                        pT[0:in2, holo:holo + hon],
                        st[0:hon, ilo2:ilo2 + in2],
                        ident_t[0:hon, 0:hon],
                        is_transpose=True,
                        start=(ch == 0), stop=(ch == NCH - 1),
                        skip_group_check=True,
                    )
                tt = tmap.tile([128, H], BF16, tag=f"T_{name}")
                if (c2 * 5 + mi) % 2 == 0:
                    nc.vector.tensor_copy(tt[0:in2, :], pT[0:in2, :])
                else:
                    nc.scalar.copy(tt[0:in2, :], pT[0:in2, :])
                t_tiles[name] = tt

            for hw in range(2):
                hsl = slice(512 * hw, 512 * hw + 512)
                p2 = {}
                for mi, name in enumerate(MAPS):
                    p2[name] = ps2.tile([128, 512], F32, tag=f"p2_{name}", name=f"p2_{name}")
                    nc.tensor.matmul(p2[name][0:n, :],
                                     _band_ap(bands_t, c2, False),
                                     t_tiles[name][0:in2, hsl],
                                     start=True, stop=True)

                # combine in fp32, reading stage-2 psum directly
                # (max one PSUM operand per instruction)
                si_sb = ctmp.tile([128, 512], F32, tag="si_sb")
                nc.scalar.copy(si_sb[0:n, :], p2["si"][0:n, :])
                P = ctmp.tile([128, 512], F32, tag="P")
                nc.vector.tensor_tensor(out=P[0:n, :], in0=si_sb[0:n, :],
                                        in1=p2["sj"][0:n, :], op=ALU.mult)
                crossN = ctmp.tile([128, 512], F32, tag="crossN")
                nc.vector.tensor_tensor(out=crossN[0:n, :],
                                        in0=p2["sij"][0:n, :],
                                        in1=P[0:n, :], op=ALU.subtract)
                # PII = si^2 in-place over si_sb (si_sb dead afterwards)
                nc.scalar.square(si_sb[0:n, :], si_sb[0:n, :])
                IvarN = ctmp.tile([128, 512], F32, tag="IvarN")
                nc.vector.tensor_tensor(out=IvarN[0:n, :],
                                        in0=p2["sii"][0:n, :],
                                        in1=si_sb[0:n, :], op=ALU.subtract)
                PJJ = ctmp.tile([128, 512], F32, tag="PJJ")
                nc.scalar.square(PJJ[0:n, :], p2["sj"][0:n, :])
                JvarN = ctmp.tile([128, 512], F32, tag="JvarN")
                nc.vector.tensor_tensor(out=JvarN[0:n, :],
                                        in0=p2["sjj"][0:n, :],
                                        in1=PJJ[0:n, :], op=ALU.subtract)
                denom = ctmp.tile([128, 512], F32, tag="denom")
                nc.vector.tensor_tensor(out=denom[0:n, :], in0=IvarN[0:n, :],
                                        in1=JvarN[0:n, :], op=ALU.mult)
                # recip = exp(-ln(denom)), in-place
                nc.scalar.activation(denom[0:n, :], denom[0:n, :], ACTF.Ln)
                nc.scalar.activation(denom[0:n, :], denom[0:n, :], ACTF.Exp,
                                     scale=-1.0)
                # c2sq in-place over crossN
                nc.scalar.square(crossN[0:n, :], crossN[0:n, :])
                nc.vector.scalar_tensor_tensor(
                    out=crossN[0:n, :], in0=crossN[0:n, :], scalar=1.0,
                    in1=denom[0:n, :], op0=ALU.mult, op1=ALU.mult,
                    accum_out=acc[0:n, c2 * 2 + hw:c2 * 2 + hw + 1])

        # ---------------- smoothness over s0, s1 ------------------------
        for ch_i, s_d in enumerate((s0_d, s1_d)):
            for t in range(8):
                st = spool.tile([128, W], F32, tag="s_in")
                eng_d = nc.sync if t % 2 == 0 else nc.scalar
                eng_d.dma_start(st[:], s_d[128 * t:128 * (t + 1), :])
                # sum s^2 (output is junk; only the accumulator matters)
                s2o = spool.tile([128, W], F32, tag="junk")
                nc.scalar.activation(s2o[:], st[:], ACTF.Square,
                                     accum_out=acc[:, 52 + ch_i * 8 + t:
                                                   53 + ch_i * 8 + t])
                # lag_w: s[w]*s[w+1]
                lw = spool.tile([128, W], F32, tag="junk")
                nc.vector.scalar_tensor_tensor(
                    out=lw[:, 0:W - 1], in0=st[:, 1:W], scalar=1.0,
                    in1=st[:, 0:W - 1], op0=ALU.mult, op1=ALU.mult,
                    accum_out=acc[:, 18 + ch_i * 8 + t:19 + ch_i * 8 + t])
                # lag_h within tile: shift rows down via SBUF->SBUF DMA
                sh = spool.tile([128, W], F32, tag="sh")
                eng_d2 = nc.scalar if t % 2 == 0 else nc.sync
                eng_d2.dma_start(sh[0:127, :], st[1:128, :])
                lh = spool.tile([128, W], F32, tag="junk")
                nc.vector.scalar_tensor_tensor(
                    out=lh[0:127, :], in0=sh[0:127, :], scalar=1.0,
                    in1=st[0:127, :], op0=ALU.mult, op1=ALU.mult,
                    accum_out=acc[0:127, 34 + ch_i * 8 + t:
                                  35 + ch_i * 8 + t])

        # ---------------- final partition reduction ---------------------
        pF = psF.tile([1, 68], F32)
        nc.tensor.matmul(pF[:], ones_t[:], acc[:], start=True, stop=True)
        outt = accp.tile([1, 68], F32, tag="outt")
        nc.scalar.copy(outt[:], pF[:])
        nc.sync.dma_start(part_d, outt[:])

    return


def _get_nc():
    if "nc" not in _nc_cache:
        nc = bass.Bass("TRN2", target_bir_lowering=False, debug=False)
        _build(nc)
        _legalize_waits(nc)
        _nc_cache["nc"] = nc
    return _nc_cache["nc"]


def kernel(I, J, s, sum_filt):
    B = I.shape[0]
    assert I.shape == (B, 1, H, W) and s.shape == (B, 2, H, W)
    nc = _get_nc()
    bands_bf, bands_f32, ident_bf, ones_f32 = _make_host_consts()

    in_maps = []
    for b in range(B):
        in_maps.append({
            "I": np.ascontiguousarray(I[b, 0]),
            "J": np.ascontiguousarray(J[b, 0]),
            "s0": np.ascontiguousarray(s[b, 0]),
            "s1": np.ascontiguousarray(s[b, 1]),
            "bands": bands_bf,
            "bandsr": bands_f32,
            "ident": ident_bf,
            "ones": ones_f32,
        })
    res = bass_utils.run_bass_kernel_spmd(nc, in_maps,
                                          core_ids=list(range(B)))
    parts = np.stack([res.results[b]["partials"][0] for b in range(B)])
    parts = parts.astype(np.float64)

    # host-side final assembly (float64)
    s64 = s.astype(np.float64)
    cc_sum = float(parts[:, 0:18].sum())
    lag_w = parts[:, 18:34].sum(axis=1)
    lag_h = parts[:, 34:52].sum(axis=1)
    s2 = parts[:, 52:68].sum(axis=1)

    # tile-boundary lag_h pairs (rows 127/128, 255/256, ...) per core
    rb = np.arange(127, H - 1, 128)
    lag_h = lag_h + (s64[:, :, rb, :] * s64[:, :, rb + 1, :]).sum(axis=(1, 2, 3))

    # edge corrections per core (both channels folded together)
    e_w = (s64[:, :, :, 0] ** 2).sum(axis=(1, 2)) + \
          (s64[:, :, :, -1] ** 2).sum(axis=(1, 2))
    e_h = (s64[:, :, 0, :] ** 2).sum(axis=(1, 2)) + \
          (s64[:, :, -1, :] ** 2).sum(axis=(1, 2))

    sum_dx2 = (2.0 * s2 - e_w - 2.0 * lag_w).sum()
    sum_dy2 = (2.0 * s2 - e_h - 2.0 * lag_h).sum()
    cnt = B * 2 * H * (W - 1)

    ncc_loss = -cc_sum / (B * H * W)
    smooth = 0.5 * (sum_dx2 / cnt + sum_dy2 / cnt) * ALPHA
    total = ncc_loss + smooth
    return np.array([total, ncc_loss, smooth], dtype=np.float32)



# revision 6
# speedup vs baseline: 1.0597x; 1.0597x over previous
"""Trainium2 Bass kernel for LocalCrossCorrelationWithSmoothnessLoss.

Full inputs in, full output out. Pure data-parallel over batch (B=8 -> 8
NeuronCores); each core computes partial sums for its image; the host
combines them into the three scalar losses.

Per-core pipeline (one 1024x1024 image pair + two flow channels):
  premaps   I,J cast to bf16 (GPSIMD); I^2,J^2 (ACT); I*J (DVE) -> 5 bf16
            maps resident in SBUF.
  stage 1   fused transpose + H-direction 9-tap box conv on the PE:
            stationary = 128x128 premap block, moving = banded H matrix
            (81-scaled for the product maps)  ->  PSUM [w, h] fp32.
            This replaces the baseline's separate transpose pass.
  T-copy    PSUM -> SBUF bf16 (DVE/ACT alternating), chunked at stride
            120 along w with the 4-wide halo baked into the chunking.
  stage 2   W-direction box conv: stationary = banded W matrix, moving =
            T chunk -> PSUM [w_out, h] fp32.
  combine   crossN = 81S_IJ - S_I*S_J, IvarN = 81S_II - S_I^2,
            JvarN = 81S_JJ - S_J^2, cc = crossN^2 * exp(-ln(IvarN*JvarN))
            read directly from PSUM, bf16 intermediates (ln in fp32),
            split across DVE/ACT/GPSIMD, accumulated per-partition.
  smooth    sum(s^2) (ACT accum), lag products (DVE STT accum; row shift
            via SBUF->SBUF DMA). Edge corrections on the host.

Output per core: 68 partial sums. Host assembles the losses in float64.
"""
import sys
import numpy as np

sys.path.insert(0, "/opt/trn_rl_repo")

import ml_dtypes
import bass_rust
import concourse.bass as bass
import concourse.tile as tile
from concourse import mybir
from concourse import bass_utils
from concourse import tile_utils

F32 = mybir.dt.float32
BF16 = mybir.dt.bfloat16
ALU = mybir.AluOpType
ACTF = mybir.ActivationFunctionType

H = 1024
W = 1024
PAD = 4
ALPHA = 0.01
STRIDE = 120
NB = 8            # h blocks of 128

# W-chunk table: (out_lo, out_n, in_lo, in_n)
WCHUNKS = []
for _c in range((W + STRIDE - 1) // STRIDE):
    _olo = STRIDE * _c
    _on = min(STRIDE, W - _olo)
    _ilo = max(0, _olo - PAD)
    _ihi = min(W, _olo + _on + PAD)
    WCHUNKS.append((_olo, _on, _ilo, _ihi - _ilo))
NWC = len(WCHUNKS)

tile_utils.max_sbuf_usage = 206 * 1024

_nc_cache = {}


def _legalize_waits(nc, max_waits=1):
    """walrus accepts only one sync-wait per instruction; split extras
    onto same-engine NoOps placed just before."""
    ctr = 0
    for f in nc.m.functions:
        for bb in f.blocks:
            insts = bb.instructions
            i = 0
            while i < len(insts):
                ins = insts[i]
                si = ins.sync_info
                if si is None:
                    i += 1
                    continue
                w = list(si.on_wait)
                if len(w) <= max_waits:
                    i += 1
                    continue
                extra, keep = w[:-max_waits], w[-max_waits:]
                nops = []
                for j in range(0, len(extra), max_waits):
                    chunk = extra[j:j + max_waits]
                    nop = mybir.InstNoOp(name=f"I-wsplit-{ctr}", ins=[], outs=[])
                    ctr += 1
                    nop.engine = ins.engine
                    nop.sync_info = bass_rust.SyncInfo(on_wait=chunk, on_update=[])
                    nops.append(nop)
                ins.sync_info = bass_rust.SyncInfo(on_wait=keep,
                                                  on_update=list(si.on_update))
                insts[i:i] = nops
                i += len(nops) + 1


def _make_host_consts():
    """bandh [128, 272] bf16 (unscaled | x81), bandw [128, 256] bf16
    (chunk-0 variant | interior variant)."""
    bh = np.zeros((128, 136), dtype=np.float32)
    h = np.arange(128)[:, None]
    j = np.arange(136)[None, :]
    bh[(h >= j - 8) & (h <= j)] = 1.0
    bandh = np.concatenate([bh, bh * 81.0], axis=1).astype(ml_dtypes.bfloat16)

    r = np.arange(128)[:, None]
    q = np.arange(128)[None, :]
    b0 = (np.abs(r - q) <= PAD).astype(np.float32)          # chunk 0
    b1 = ((r >= q) & (r <= q + 8)).astype(np.float32)       # interior
    bandw = np.concatenate([b0, b1], axis=1).astype(ml_dtypes.bfloat16)
    return {"bandh": bandh, "bandw": bandw}


def _fused_mm_list(hh):
    """MM descriptors for one psum half tile (h_out in [512*hh, 512*hh+511]).
    Returns list of (h_block, j_lo, j_n, psum_col)."""
    mms = []
    for b in range(4 * hh, 4 * hh + 4):
        base = 128 * b - 4
        lo = max(512 * hh, base)
        hi = min(512 * hh + 511, base + 135)
        mms.append((b, lo - base, hi - lo + 1, lo - 512 * hh))
    if hh == 1:
        b = 4 * hh - 1
        base = 128 * b - 4
        mms.append((b, 512 * hh - base, 4, 0))
    if hh == 0:
        b = 4
        base = 128 * b - 4
        mms.append((b, 0, 4, base - 512 * hh))
    return mms


def _build(nc):
    I_d = nc.dram_tensor("I", [H, W], F32, kind="ExternalInput").ap()
    J_d = nc.dram_tensor("J", [H, W], F32, kind="ExternalInput").ap()
    s0_d = nc.dram_tensor("s0", [H, W], F32, kind="ExternalInput").ap()
    s1_d = nc.dram_tensor("s1", [H, W], F32, kind="ExternalInput").ap()
    bandh_d = nc.dram_tensor("bandh", [128, 272], BF16,
                             kind="ExternalInput").ap()
    bandw_d = nc.dram_tensor("bandw", [128, 256], BF16,
                             kind="ExternalInput").ap()
    part_d = nc.dram_tensor("partials", [128, 68], F32,
                            kind="ExternalOutput").ap()

    MAPS = ("si", "sj", "sij", "sii", "sjj")

    from contextlib import ExitStack
    with tile.TileContext(nc) as tc, ExitStack() as ctx:
        consts = ctx.enter_context(tc.tile_pool(name="consts", bufs=1))
        inp = ctx.enter_context(tc.tile_pool(name="inp", bufs=2))
        pmap = ctx.enter_context(tc.tile_pool(name="pmap", bufs=1))
        tmap = ctx.enter_context(tc.tile_pool(name="tmap", bufs=2))
        ctmp = ctx.enter_context(tc.tile_pool(name="ctmp", bufs=2))
        spool = ctx.enter_context(tc.tile_pool(name="spool", bufs=2))
        jpool = ctx.enter_context(tc.tile_pool(name="jpool", bufs=4))
        accp = ctx.enter_context(tc.tile_pool(name="accp", bufs=1))
        psA = ctx.enter_context(tc.tile_pool(name="psA", bufs=3, space="PSUM"))
        ps2 = ctx.enter_context(tc.tile_pool(name="ps2", bufs=1, space="PSUM"))

        bandh_t = consts.tile([128, 272], BF16)
        bandw_t = consts.tile([128, 256], BF16)
        nc.sync.dma_start(bandh_t[:], bandh_d)
        nc.sync.dma_start(bandw_t[:], bandw_d)

        # accumulators: accum_out OVERWRITES, so every accumulating
        # instruction gets its own column; host sums the groups.
        # cols 0-17: cc per (wchunk,half); 18-33: lag_w; 34-49: lag_h;
        # 52-67: s^2  (50-51 unused; layout matches host assembly)
        acc = accp.tile([128, 68], F32)
        nc.vector.memset(acc[:], 0.0)

        # ---------------- premaps: 5 bf16 maps per h-block ---------------
        pm = {}
        smooth_jobs = []
        for ch_i, s_d in enumerate((s0_d, s1_d)):
            for t in range(8):
                smooth_jobs.append((ch_i, t, s_d))

        def emit_smooth(job):
            ch_i, t, s_d = job
            st = spool.tile([128, W], F32, tag="s_in")
            eng_d = nc.sync if t % 2 == 0 else nc.scalar
            eng_d.dma_start(st[:], s_d[128 * t:128 * (t + 1), :])
            # sum s^2 (output is junk; only the accumulator matters)
            s2o = jpool.tile([128, W], F32, tag="junk")
            nc.scalar.activation(s2o[:], st[:], ACTF.Square,
                                 accum_out=acc[:, 52 + ch_i * 8 + t:
                                               53 + ch_i * 8 + t])
            # lag_w: s[w]*s[w+1]
            lw = jpool.tile([128, W], F32, tag="junk")
            nc.vector.scalar_tensor_tensor(
                out=lw[:, 0:W - 1], in0=st[:, 1:W], scalar=1.0,
                in1=st[:, 0:W - 1], op0=ALU.mult, op1=ALU.mult,
                accum_out=acc[:, 18 + ch_i * 8 + t:19 + ch_i * 8 + t])
            # lag_h within tile: shift rows down via SBUF->SBUF DMA
            sh = spool.tile([128, W], F32, tag="sh")
            eng_d2 = nc.scalar if t % 2 == 0 else nc.sync
            eng_d2.dma_start(sh[0:127, :], st[1:128, :])
            lh = jpool.tile([128, W], F32, tag="junk")
            nc.vector.scalar_tensor_tensor(
                out=lh[0:127, :], in0=sh[0:127, :], scalar=1.0,
                in1=st[0:127, :], op0=ALU.mult, op1=ALU.mult,
                accum_out=acc[0:127, 34 + ch_i * 8 + t:
                              35 + ch_i * 8 + t])

        for b in range(NB):
            I_t = inp.tile([128, W], F32, tag="I_in")
            J_t = inp.tile([128, W], F32, tag="J_in")
            nc.sync.dma_start(I_t[:], I_d[128 * b:128 * (b + 1), :])
            nc.scalar.dma_start(J_t[:], J_d[128 * b:128 * (b + 1), :])
            for name in MAPS:
                pm[(name, b)] = pmap.tile([128, W], BF16,
                                          tag=f"pm_{name}_{b}",
                                          name=f"pm_{name}_{b}")
            nc.gpsimd.tensor_copy(pm[("si", b)][:], I_t[:])
            nc.gpsimd.tensor_copy(pm[("sj", b)][:], J_t[:])
            nc.scalar.square(pm[("sii", b)][:], I_t[:])
            nc.scalar.square(pm[("sjj", b)][:], J_t[:])
            nc.vector.tensor_tensor(out=pm[("sij", b)][:], in0=I_t[:],
                                    in1=J_t[:], op=ALU.mult)
            # interleave two smoothness tiles per h-block
            for _ in range(2):
                if smooth_jobs:
                    emit_smooth(smooth_jobs.pop(0))

        # ------------- per w-chunk: fused H-conv+transpose, W-conv, -----
        # ------------- combine ------------------------------------------
        for c, (olo, on, ilo, inn) in enumerate(WCHUNKS):
            t_tiles = {}
            for mi, name in enumerate(MAPS):
                scaled = mi >= 2
                bh_off = 136 if scaled else 0
                tt = tmap.tile([128, H], BF16, tag=f"T_{name}")
                t_tiles[name] = tt
                for hh in range(2):
                    pT = psA.tile([128, 512], F32, tag="psA")
                    mms = _fused_mm_list(hh)
                    for k, (b, jlo, jn, pcol) in enumerate(mms):
                        nc.tensor.matmul(
                            pT[0:inn, pcol:pcol + jn],
                            pm[(name, b)][:, ilo:ilo + inn],
                            bandh_t[:, bh_off + jlo:bh_off + jlo + jn],
                            start=(k == 0), stop=(k == len(mms) - 1),
                            skip_group_check=True,
                        )
                    if (mi * 2 + hh + c) % 2 == 0:
                        nc.vector.tensor_copy(
                            tt[0:inn, 512 * hh:512 * hh + 512], pT[0:inn, :])
                    else:
                        nc.scalar.copy(
                            tt[0:inn, 512 * hh:512 * hh + 512], pT[0:inn, :])

            bw_off = 0 if c == 0 else 128
            for hh in range(2):
                hsl = slice(512 * hh, 512 * hh + 512)
                p2 = {}
                for name in MAPS:
                    p2[name] = ps2.tile([128, 512], F32, tag=f"p2_{name}",
                                        name=f"p2_{name}_{c}_{hh}")
                    nc.tensor.matmul(p2[name][:, :],
                                     bandw_t[0:inn, bw_off:bw_off + 128],
                                     t_tiles[name][0:inn, hsl],
                                     start=True, stop=True)

                n = on
                # combine; bf16 intermediates, ln in fp32
                si_sb = ctmp.tile([128, 512], BF16, tag="si_sb")
                nc.scalar.copy(si_sb[0:n, :], p2["si"][0:n, :])
                P = ctmp.tile([128, 512], BF16, tag="P")
                nc.vector.scalar_tensor_tensor(
                    out=P[0:n, :], in0=si_sb[0:n, :], scalar=1.0,
                    in1=p2["sj"][0:n, :], op0=ALU.mult, op1=ALU.mult)
                crossN = ctmp.tile([128, 512], BF16, tag="crossN")
                nc.vector.scalar_tensor_tensor(
                    out=crossN[0:n, :], in0=p2["sij"][0:n, :], scalar=1.0,
                    in1=P[0:n, :], op0=ALU.mult, op1=ALU.subtract)
                si2 = ctmp.tile([128, 512], BF16, tag="si2")
                nc.gpsimd.tensor_tensor(out=si2[0:n, :], in0=si_sb[0:n, :],
                                        in1=si_sb[0:n, :], op=ALU.mult)
                IvarN = ctmp.tile([128, 512], BF16, tag="IvarN")
                nc.vector.scalar_tensor_tensor(
                    out=IvarN[0:n, :], in0=p2["sii"][0:n, :], scalar=1.0,
                    in1=si2[0:n, :], op0=ALU.mult, op1=ALU.subtract)
                sj2 = ctmp.tile([128, 512], BF16, tag="sj2")
                nc.scalar.square(sj2[0:n, :], p2["sj"][0:n, :])
                JvarN = ctmp.tile([128, 512], BF16, tag="JvarN")
                nc.vector.scalar_tensor_tensor(
                    out=JvarN[0:n, :], in0=p2["sjj"][0:n, :], scalar=1.0,
                    in1=sj2[0:n, :], op0=ALU.mult, op1=ALU.subtract)
                denom = ctmp.tile([128, 512], BF16, tag="denom")
                nc.gpsimd.tensor_tensor(out=denom[0:n, :], in0=IvarN[0:n, :],
                                        in1=JvarN[0:n, :], op=ALU.mult)
                lnd = ctmp.tile([128, 512], F32, tag="lnd")
                nc.scalar.activation(lnd[0:n, :], denom[0:n, :], ACTF.Ln)
                recip = ctmp.tile([128, 512], BF16, tag="recip")
                nc.scalar.activation(recip[0:n, :], lnd[0:n, :], ACTF.Exp,
                                     scale=-1.0)
                cross2 = ctmp.tile([128, 512], BF16, tag="cross2")
                nc.gpsimd.tensor_tensor(out=cross2[0:n, :],
                                        in0=crossN[0:n, :],
                                        in1=crossN[0:n, :], op=ALU.mult)
                ccj = ctmp.tile([128, 512], BF16, tag="ccj")
                nc.vector.scalar_tensor_tensor(
                    out=ccj[0:n, :], in0=cross2[0:n, :], scalar=1.0,
                    in1=recip[0:n, :], op0=ALU.mult, op1=ALU.mult,
                    accum_out=acc[0:n, c * 2 + hh:c * 2 + hh + 1])

        # final partition reduction happens on the host (float64)
        nc.sync.dma_start(part_d, acc[:])

    return


def _get_nc():
    if "nc" not in _nc_cache:
        nc = bass.Bass("TRN2", target_bir_lowering=False, debug=False)
        _build(nc)
        _legalize_waits(nc)
        _nc_cache["nc"] = nc
    return _nc_cache["nc"]


def _make_in_maps(I, J, s):
    B = I.shape[0]
    consts = _make_host_consts()
    in_maps = []
    for b in range(B):
        m = {
            "I": np.ascontiguousarray(I[b, 0]),
            "J": np.ascontiguousarray(J[b, 0]),
            "s0": np.ascontiguousarray(s[b, 0]),
            "s1": np.ascontiguousarray(s[b, 1]),
        }
        m.update(consts)
        in_maps.append(m)
    return in_maps


def kernel(I, J, s, sum_filt):
    B = I.shape[0]
    assert I.shape == (B, 1, H, W) and s.shape == (B, 2, H, W)
    nc = _get_nc()
    in_maps = _make_in_maps(I, J, s)
    res = bass_utils.run_bass_kernel_spmd(nc, in_maps,
                                          core_ids=list(range(B)))
    parts = np.stack([res.results[b]["partials"] for b in range(B)])
    parts = parts.astype(np.float64).sum(axis=1)   # reduce partition dim

    # host-side final assembly (float64)
    s64 = s.astype(np.float64)
    cc_sum = float(parts[:, 0:18].sum())
    lag_w = parts[:, 18:34].sum(axis=1)
    lag_h = parts[:, 34:52].sum(axis=1)
    s2 = parts[:, 52:68].sum(axis=1)

    # tile-boundary lag_h pairs (rows 127/128, 255/256, ...) per core
    rb = np.arange(127, H - 1, 128)
    lag_h = lag_h + (s64[:, :, rb, :] * s64[:, :, rb + 1, :]).sum(axis=(1, 2, 3))

    # edge corrections per core (both channels folded together)
    e_w = (s64[:, :, :, 0] ** 2).sum(axis=(1, 2)) + \
          (s64[:, :, :, -1] ** 2).sum(axis=(1, 2))
    e_h = (s64[:, :, 0, :] ** 2).sum(axis=(1, 2)) + \
          (s64[:, :, -1, :] ** 2).sum(axis=(1, 2))

    sum_dx2 = (2.0 * s2 - e_w - 2.0 * lag_w).sum()
    sum_dy2 = (2.0 * s2 - e_h - 2.0 * lag_h).sum()
    cnt = B * 2 * H * (W - 1)

    ncc_loss = -cc_sum / (B * H * W)
    smooth = 0.5 * (sum_dx2 / cnt + sum_dy2 / cnt) * ALPHA
    total = ncc_loss + smooth
    return np.array([total, ncc_loss, smooth], dtype=np.float32)


# revision 8
# speedup vs baseline: 2.0128x; 1.8993x over previous
"""Trainium2 Bass kernel for LocalCrossCorrelationWithSmoothnessLoss.

Full inputs in, full output out. Pure data-parallel over batch (B=8 -> 8
NeuronCores); each core computes partial sums for its image; the host
combines them into the three scalar losses.

Per-core pipeline (one 1024x1024 image pair + two flow channels):
  premaps   I,J cast to bf16 (GPSIMD); I^2,J^2 (ACT); I*J (DVE) -> 5 bf16
            maps resident in SBUF.
  stage 1   fused transpose + H-direction 9-tap box conv on the PE:
            stationary = 128x128 premap block, moving = banded H matrix
            (81-scaled for the product maps)  ->  PSUM [w, h] fp32.
            This replaces the baseline's separate transpose pass.
  T-copy    PSUM -> SBUF bf16 (DVE/ACT alternating), chunked at stride
            120 along w with the 4-wide halo baked into the chunking.
  stage 2   W-direction box conv: stationary = banded W matrix, moving =
            T chunk -> PSUM [w_out, h] fp32.
  combine   crossN = 81S_IJ - S_I*S_J, IvarN = 81S_II - S_I^2,
            JvarN = 81S_JJ - S_J^2, cc = crossN^2 * exp(-ln(IvarN*JvarN))
            read directly from PSUM, bf16 intermediates (ln in fp32),
            split across DVE/ACT/GPSIMD, accumulated per-partition.
  smooth    sum(s^2) (ACT accum), lag products (DVE STT accum; row shift
            via SBUF->SBUF DMA). Edge corrections on the host.

Output per core: 68 partial sums. Host assembles the losses in float64.
"""
import sys
import numpy as np

sys.path.insert(0, "/opt/trn_rl_repo")

import ml_dtypes
import bass_rust
import concourse.bass as bass
import concourse.tile as tile
from concourse import mybir
from concourse import bass_utils
from concourse import tile_utils

F32 = mybir.dt.float32
BF16 = mybir.dt.bfloat16
ALU = mybir.AluOpType
ACTF = mybir.ActivationFunctionType

H = 1024
W = 1024
PAD = 4
ALPHA = 0.01
STRIDE = 120
NB = 8            # h blocks of 128

# W-chunk table: (out_lo, out_n, in_lo, in_n)
WCHUNKS = []
for _c in range((W + STRIDE - 1) // STRIDE):
    _olo = STRIDE * _c
    _on = min(STRIDE, W - _olo)
    _ilo = max(0, _olo - PAD)
    _ihi = min(W, _olo + _on + PAD)
    WCHUNKS.append((_olo, _on, _ilo, _ihi - _ilo))
NWC = len(WCHUNKS)

tile_utils.max_sbuf_usage = 206 * 1024

_nc_cache = {}


def _legalize_waits(nc, max_waits=1):
    """walrus accepts only one sync-wait per instruction; split extras
    onto same-engine NoOps placed just before."""
    ctr = 0
    for f in nc.m.functions:
        for bb in f.blocks:
            insts = bb.instructions
            i = 0
            while i < len(insts):
                ins = insts[i]
                si = ins.sync_info
                if si is None:
                    i += 1
                    continue
                w = list(si.on_wait)
                if len(w) <= max_waits:
                    i += 1
                    continue
                extra, keep = w[:-max_waits], w[-max_waits:]
                nops = []
                for j in range(0, len(extra), max_waits):
                    chunk = extra[j:j + max_waits]
                    nop = mybir.InstNoOp(name=f"I-wsplit-{ctr}", ins=[], outs=[])
                    ctr += 1
                    nop.engine = ins.engine
                    nop.sync_info = bass_rust.SyncInfo(on_wait=chunk, on_update=[])
                    nops.append(nop)
                ins.sync_info = bass_rust.SyncInfo(on_wait=keep,
                                                  on_update=list(si.on_update))
                insts[i:i] = nops
                i += len(nops) + 1


def _make_host_consts():
    """bandh [128, 272] bf16 (unscaled | x81), bandw [128, 256] bf16
    (chunk-0 variant | interior variant)."""
    bh = np.zeros((128, 136), dtype=np.float32)
    h = np.arange(128)[:, None]
    j = np.arange(136)[None, :]
    bh[(h >= j - 8) & (h <= j)] = 1.0
    bandh = np.concatenate([bh, bh * 81.0], axis=1).astype(ml_dtypes.bfloat16)

    r = np.arange(128)[:, None]
    q = np.arange(128)[None, :]
    b0 = (np.abs(r - q) <= PAD).astype(np.float32)          # chunk 0
    b1 = ((r >= q) & (r <= q + 8)).astype(np.float32)       # interior
    bandw = np.concatenate([b0, b1], axis=1).astype(ml_dtypes.bfloat16)
    return {"bandh": bandh, "bandw": bandw}


def _fused_mm_list(hh):
    """MM descriptors for one psum half tile (h_out in [512*hh, 512*hh+511]).
    Returns list of (h_block, j_lo, j_n, psum_col)."""
    mms = []
    for b in range(4 * hh, 4 * hh + 4):
        base = 128 * b - 4
        lo = max(512 * hh, base)
        hi = min(512 * hh + 511, base + 135)
        mms.append((b, lo - base, hi - lo + 1, lo - 512 * hh))
    if hh == 1:
        b = 4 * hh - 1
        base = 128 * b - 4
        mms.append((b, 512 * hh - base, 4, 0))
    if hh == 0:
        b = 4
        base = 128 * b - 4
        mms.append((b, 0, 4, base - 512 * hh))
    return mms


def _build(nc):
    I_d = nc.dram_tensor("I", [H, W], F32, kind="ExternalInput").ap()
    J_d = nc.dram_tensor("J", [H, W], F32, kind="ExternalInput").ap()
    s0_d = nc.dram_tensor("s0", [H, W], F32, kind="ExternalInput").ap()
    s1_d = nc.dram_tensor("s1", [H, W], F32, kind="ExternalInput").ap()
    bandh_d = nc.dram_tensor("bandh", [128, 272], BF16,
                             kind="ExternalInput").ap()
    bandw_d = nc.dram_tensor("bandw", [128, 256], BF16,
                             kind="ExternalInput").ap()
    part_d = nc.dram_tensor("partials", [128, 68], F32,
                            kind="ExternalOutput").ap()

    MAPS = ("si", "sj", "sij", "sii", "sjj")

    from contextlib import ExitStack
    with tile.TileContext(nc) as tc, ExitStack() as ctx:
        consts = ctx.enter_context(tc.tile_pool(name="consts", bufs=1))
        inp = ctx.enter_context(tc.tile_pool(name="inp", bufs=2))
        pmap = ctx.enter_context(tc.tile_pool(name="pmap", bufs=1))
        tmap = ctx.enter_context(tc.tile_pool(name="tmap", bufs=2))
        ctmp = ctx.enter_context(tc.tile_pool(name="ctmp", bufs=2))
        spool = ctx.enter_context(tc.tile_pool(name="spool", bufs=2))
        jpool = ctx.enter_context(tc.tile_pool(name="jpool", bufs=4))
        accp = ctx.enter_context(tc.tile_pool(name="accp", bufs=1))
        psA = ctx.enter_context(tc.tile_pool(name="psA", bufs=3, space="PSUM"))
        ps2 = ctx.enter_context(tc.tile_pool(name="ps2", bufs=1, space="PSUM"))

        bandh_t = consts.tile([128, 272], BF16)
        bandw_t = consts.tile([128, 256], BF16)
        nc.sync.dma_start(bandh_t[:], bandh_d)
        nc.sync.dma_start(bandw_t[:], bandw_d)

        # accumulators: accum_out OVERWRITES, so every accumulating
        # instruction gets its own column; host sums the groups.
        # cols 0-17: cc per (wchunk,half); 18-33: lag_w; 34-49: lag_h;
        # 52-67: s^2  (50-51 unused; layout matches host assembly)
        acc = accp.tile([128, 68], F32)
        nc.vector.memset(acc[:], 0.0)

        # ---------------- premaps: 5 bf16 maps per h-block ---------------
        pm = {}
        smooth_jobs = []
        for ch_i, s_d in enumerate((s0_d, s1_d)):
            for t in range(8):
                smooth_jobs.append((ch_i, t, s_d))

        def emit_smooth(job):
            ch_i, t, s_d = job
            st = spool.tile([128, W], F32, tag="s_in")
            eng_d = nc.sync if t % 2 == 0 else nc.scalar
            eng_d.dma_start(st[:], s_d[128 * t:128 * (t + 1), :])
            # sum s^2 (output is junk; only the accumulator matters)
            s2o = jpool.tile([128, W], F32, tag="junk")
            nc.scalar.activation(s2o[:], st[:], ACTF.Square,
                                 accum_out=acc[:, 52 + ch_i * 8 + t:
                                               53 + ch_i * 8 + t])
            # lag_w: s[w]*s[w+1]
            lw = jpool.tile([128, W], F32, tag="junk")
            nc.vector.scalar_tensor_tensor(
                out=lw[:, 0:W - 1], in0=st[:, 1:W], scalar=1.0,
                in1=st[:, 0:W - 1], op0=ALU.mult, op1=ALU.mult,
                accum_out=acc[:, 18 + ch_i * 8 + t:19 + ch_i * 8 + t])
            # lag_h: row-shifted copy loaded straight from DRAM (row t*128+1
            # onward), so s[h]*s[h+1] covers tile boundaries too
            nsh = 128 if t < 7 else 127
            sh = spool.tile([128, W], F32, tag="sh")
            eng_d2 = nc.scalar if t % 2 == 0 else nc.sync
            eng_d2.dma_start(sh[0:nsh, :],
                             s_d[128 * t + 1:128 * t + 1 + nsh, :])
            lh = jpool.tile([128, W], F32, tag="junk")
            nc.vector.scalar_tensor_tensor(
                out=lh[0:nsh, :], in0=sh[0:nsh, :], scalar=1.0,
                in1=st[0:nsh, :], op0=ALU.mult, op1=ALU.mult,
                accum_out=acc[0:nsh, 34 + ch_i * 8 + t:
                              35 + ch_i * 8 + t])

        for b in range(NB):
            I_t = inp.tile([128, W], F32, tag="I_in")
            J_t = inp.tile([128, W], F32, tag="J_in")
            nc.sync.dma_start(I_t[:], I_d[128 * b:128 * (b + 1), :])
            nc.scalar.dma_start(J_t[:], J_d[128 * b:128 * (b + 1), :])
            for name in MAPS:
                pm[(name, b)] = pmap.tile([128, W], BF16,
                                          tag=f"pm_{name}_{b}",
                                          name=f"pm_{name}_{b}")
            nc.gpsimd.tensor_copy(pm[("si", b)][:], I_t[:])
            nc.gpsimd.tensor_copy(pm[("sj", b)][:], J_t[:])
            nc.scalar.square(pm[("sii", b)][:], I_t[:])
            nc.scalar.square(pm[("sjj", b)][:], J_t[:])
            nc.vector.tensor_tensor(out=pm[("sij", b)][:], in0=I_t[:],
                                    in1=J_t[:], op=ALU.mult)
            # interleave two smoothness tiles per h-block
            for _ in range(2):
                if smooth_jobs:
                    emit_smooth(smooth_jobs.pop(0))

        # ------------- per w-chunk: fused H-conv+transpose, W-conv, -----
        # ------------- combine ------------------------------------------
        for c, (olo, on, ilo, inn) in enumerate(WCHUNKS):
            t_tiles = {}
            for mi, name in enumerate(MAPS):
                scaled = mi >= 2
                bh_off = 136 if scaled else 0
                tt = tmap.tile([128, H], BF16, tag=f"T_{name}")
                t_tiles[name] = tt
                for hh in range(2):
                    pT = psA.tile([128, 512], F32, tag="psA")
                    mms = _fused_mm_list(hh)
                    for k, (b, jlo, jn, pcol) in enumerate(mms):
                        nc.tensor.matmul(
                            pT[0:inn, pcol:pcol + jn],
                            pm[(name, b)][:, ilo:ilo + inn],
                            bandh_t[:, bh_off + jlo:bh_off + jlo + jn],
                            start=(k == 0), stop=(k == len(mms) - 1),
                            skip_group_check=True,
                        )
                    if (mi * 2 + hh + c) % 2 == 0:
                        nc.vector.tensor_copy(
                            tt[0:inn, 512 * hh:512 * hh + 512], pT[0:inn, :])
                    else:
                        nc.scalar.copy(
                            tt[0:inn, 512 * hh:512 * hh + 512], pT[0:inn, :])

            bw_off = 0 if c == 0 else 128
            for hh in range(2):
                hsl = slice(512 * hh, 512 * hh + 512)
                p2 = {}
                for name in MAPS:
                    p2[name] = ps2.tile([128, 512], F32, tag=f"p2_{name}",
                                        name=f"p2_{name}_{c}_{hh}")
                    nc.tensor.matmul(p2[name][:, :],
                                     bandw_t[0:inn, bw_off:bw_off + 128],
                                     t_tiles[name][0:inn, hsl],
                                     start=True, stop=True)

                n = on
                # combine; bf16 intermediates, ln in fp32
                si_sb = ctmp.tile([128, 512], BF16, tag="si_sb")
                nc.scalar.copy(si_sb[0:n, :], p2["si"][0:n, :])
                P = ctmp.tile([128, 512], BF16, tag="P")
                nc.vector.scalar_tensor_tensor(
                    out=P[0:n, :], in0=si_sb[0:n, :], scalar=1.0,
                    in1=p2["sj"][0:n, :], op0=ALU.mult, op1=ALU.mult)
                crossN = ctmp.tile([128, 512], BF16, tag="crossN")
                nc.vector.scalar_tensor_tensor(
                    out=crossN[0:n, :], in0=p2["sij"][0:n, :], scalar=1.0,
                    in1=P[0:n, :], op0=ALU.mult, op1=ALU.subtract)
                si2 = ctmp.tile([128, 512], BF16, tag="si2")
                nc.gpsimd.tensor_tensor(out=si2[0:n, :], in0=si_sb[0:n, :],
                                        in1=si_sb[0:n, :], op=ALU.mult)
                IvarN = ctmp.tile([128, 512], BF16, tag="IvarN")
                nc.vector.scalar_tensor_tensor(
                    out=IvarN[0:n, :], in0=p2["sii"][0:n, :], scalar=1.0,
                    in1=si2[0:n, :], op0=ALU.mult, op1=ALU.subtract)
                sj2 = ctmp.tile([128, 512], BF16, tag="sj2")
                nc.scalar.square(sj2[0:n, :], p2["sj"][0:n, :])
                JvarN = ctmp.tile([128, 512], BF16, tag="JvarN")
                nc.vector.scalar_tensor_tensor(
                    out=JvarN[0:n, :], in0=p2["sjj"][0:n, :], scalar=1.0,
                    in1=sj2[0:n, :], op0=ALU.mult, op1=ALU.subtract)
                denom = ctmp.tile([128, 512], BF16, tag="denom")
                nc.gpsimd.tensor_tensor(out=denom[0:n, :], in0=IvarN[0:n, :],
                                        in1=JvarN[0:n, :], op=ALU.mult)
                lnd = ctmp.tile([128, 512], F32, tag="lnd")
                nc.scalar.activation(lnd[0:n, :], denom[0:n, :], ACTF.Ln)
                recip = ctmp.tile([128, 512], BF16, tag="recip")
                nc.scalar.activation(recip[0:n, :], lnd[0:n, :], ACTF.Exp,
                                     scale=-1.0)
                cross2 = ctmp.tile([128, 512], BF16, tag="cross2")
                nc.gpsimd.tensor_tensor(out=cross2[0:n, :],
                                        in0=crossN[0:n, :],
                                        in1=crossN[0:n, :], op=ALU.mult)
                ccj = ctmp.tile([128, 512], BF16, tag="ccj")
                nc.vector.scalar_tensor_tensor(
                    out=ccj[0:n, :], in0=cross2[0:n, :], scalar=1.0,
                    in1=recip[0:n, :], op0=ALU.mult, op1=ALU.mult,
                    accum_out=acc[0:n, c * 2 + hh:c * 2 + hh + 1])

        # final partition reduction happens on the host (float64)
        nc.sync.dma_start(part_d, acc[:])

    return


def _get_nc():
    if "nc" not in _nc_cache:
        nc = bass.Bass("TRN2", target_bir_lowering=False, debug=False)
        _build(nc)
        _legalize_waits(nc)
        _nc_cache["nc"] = nc
    return _nc_cache["nc"]


def _make_in_maps(I, J, s):
    B = I.shape[0]
    consts = _make_host_consts()
    in_maps = []
    for b in range(B):
        m = {
            "I": np.ascontiguousarray(I[b, 0]),
            "J": np.ascontiguousarray(J[b, 0]),
            "s0": np.ascontiguousarray(s[b, 0]),
            "s1": np.ascontiguousarray(s[b, 1]),
        }
        m.update(consts)
        in_maps.append(m)
    return in_maps


def kernel(I, J, s, sum_filt):
    B = I.shape[0]
    assert I.shape == (B, 1, H, W) and s.shape == (B, 2, H, W)
    nc = _get_nc()
    in_maps = _make_in_maps(I, J, s)
    res = bass_utils.run_bass_kernel_spmd(nc, in_maps,
                                          core_ids=list(range(B)))
    parts = np.stack([res.results[b]["partials"] for b in range(B)])
    parts = parts.astype(np.float64).sum(axis=1)   # reduce partition dim

    # host-side final assembly (float64)
    s64 = s.astype(np.float64)
    cc_sum = float(parts[:, 0:18].sum())
    lag_w = parts[:, 18:34].sum(axis=1)
    lag_h = parts[:, 34:52].sum(axis=1)
    s2 = parts[:, 52:68].sum(axis=1)

    # edge corrections per core (both channels folded together)
    e_w = (s64[:, :, :, 0] ** 2).sum(axis=(1, 2)) + \
          (s64[:, :, :, -1] ** 2).sum(axis=(1, 2))
    e_h = (s64[:, :, 0, :] ** 2).sum(axis=(1, 2)) + \
          (s64[:, :, -1, :] ** 2).sum(axis=(1, 2))

    sum_dx2 = (2.0 * s2 - e_w - 2.0 * lag_w).sum()
    sum_dy2 = (2.0 * s2 - e_h - 2.0 * lag_h).sum()
    cnt = B * 2 * H * (W - 1)

    ncc_loss = -cc_sum / (B * H * W)
    smooth = 0.5 * (sum_dx2 / cnt + sum_dy2 / cnt) * ALPHA
    total = ncc_loss + smooth
    return np.array([total, ncc_loss, smooth], dtype=np.float32)


# revision 9
# speedup vs baseline: 2.0790x; 1.0329x over previous
"""Trainium2 Bass kernel for LocalCrossCorrelationWithSmoothnessLoss.

Full inputs in, full output out. Pure data-parallel over batch (B=8 -> 8
NeuronCores); each core computes partial sums for its image; the host
combines them into the three scalar losses.

Per-core pipeline (one 1024x1024 image pair + two flow channels):
  premaps   I,J cast to bf16 (GPSIMD); I^2,J^2 (ACT); I*J (DVE) -> 5 bf16
            maps resident in SBUF.
  stage 1   fused transpose + H-direction 9-tap box conv on the PE:
            stationary = 128x128 premap block, moving = banded H matrix
            (81-scaled for the product maps)  ->  PSUM [w, h] fp32.
            This replaces the baseline's separate transpose pass.
  T-copy    PSUM -> SBUF bf16 (DVE/ACT alternating), chunked at stride
            120 along w with the 4-wide halo baked into the chunking.
  stage 2   W-direction box conv: stationary = banded W matrix, moving =
            T chunk -> PSUM [w_out, h] fp32.
  combine   crossN = 81S_IJ - S_I*S_J, IvarN = 81S_II - S_I^2,
            JvarN = 81S_JJ - S_J^2, cc = crossN^2 * exp(-ln(IvarN*JvarN))
            read directly from PSUM, bf16 intermediates (ln in fp32),
            split across DVE/ACT/GPSIMD, accumulated per-partition.
  smooth    sum(s^2) (ACT accum), lag products (DVE STT accum; row shift
            via SBUF->SBUF DMA). Edge corrections on the host.

Output per core: 68 partial sums. Host assembles the losses in float64.
"""
import sys
import numpy as np

sys.path.insert(0, "/opt/trn_rl_repo")

import ml_dtypes
import bass_rust
import concourse.bass as bass
import concourse.tile as tile
from concourse import mybir
from concourse import bass_utils
from concourse import tile_utils

F32 = mybir.dt.float32
BF16 = mybir.dt.bfloat16
ALU = mybir.AluOpType
ACTF = mybir.ActivationFunctionType

H = 1024
W = 1024
PAD = 4
ALPHA = 0.01
STRIDE = 120
NB = 8            # h blocks of 128

# W-chunk table: (out_lo, out_n, in_lo, in_n)
WCHUNKS = []
for _c in range((W + STRIDE - 1) // STRIDE):
    _olo = STRIDE * _c
    _on = min(STRIDE, W - _olo)
    _ilo = max(0, _olo - PAD)
    _ihi = min(W, _olo + _on + PAD)
    WCHUNKS.append((_olo, _on, _ilo, _ihi - _ilo))
NWC = len(WCHUNKS)

tile_utils.max_sbuf_usage = 206 * 1024

_nc_cache = {}


def _legalize_waits(nc, max_waits=1):
    """walrus accepts only one sync-wait per instruction; split extras
    onto same-engine NoOps placed just before."""
    ctr = 0
    for f in nc.m.functions:
        for bb in f.blocks:
            insts = bb.instructions
            i = 0
            while i < len(insts):
                ins = insts[i]
                si = ins.sync_info
                if si is None:
                    i += 1
                    continue
                w = list(si.on_wait)
                if len(w) <= max_waits:
                    i += 1
                    continue
                extra, keep = w[:-max_waits], w[-max_waits:]
                nops = []
                for j in range(0, len(extra), max_waits):
                    chunk = extra[j:j + max_waits]
                    nop = mybir.InstNoOp(name=f"I-wsplit-{ctr}", ins=[], outs=[])
                    ctr += 1
                    nop.engine = ins.engine
                    nop.sync_info = bass_rust.SyncInfo(on_wait=chunk, on_update=[])
                    nops.append(nop)
                ins.sync_info = bass_rust.SyncInfo(on_wait=keep,
                                                  on_update=list(si.on_update))
                insts[i:i] = nops
                i += len(nops) + 1


def _make_host_consts():
    """bandh [128, 272] bf16 (unscaled | x81), bandw [128, 256] bf16
    (chunk-0 variant | interior variant)."""
    bh = np.zeros((128, 136), dtype=np.float32)
    h = np.arange(128)[:, None]
    j = np.arange(136)[None, :]
    bh[(h >= j - 8) & (h <= j)] = 1.0
    bandh = np.concatenate([bh, bh * 81.0], axis=1).astype(ml_dtypes.bfloat16)

    r = np.arange(128)[:, None]
    q = np.arange(128)[None, :]
    b0 = (np.abs(r - q) <= PAD).astype(np.float32)          # chunk 0
    b1 = ((r >= q) & (r <= q + 8)).astype(np.float32)       # interior
    bandw = np.concatenate([b0, b1], axis=1).astype(ml_dtypes.bfloat16)
    return {"bandh": bandh, "bandw": bandw}


def _fused_mm_list(hh):
    """MM descriptors for one psum half tile (h_out in [512*hh, 512*hh+511]).
    Returns list of (h_block, j_lo, j_n, psum_col)."""
    mms = []
    for b in range(4 * hh, 4 * hh + 4):
        base = 128 * b - 4
        lo = max(512 * hh, base)
        hi = min(512 * hh + 511, base + 135)
        mms.append((b, lo - base, hi - lo + 1, lo - 512 * hh))
    if hh == 1:
        b = 4 * hh - 1
        base = 128 * b - 4
        mms.append((b, 512 * hh - base, 4, 0))
    if hh == 0:
        b = 4
        base = 128 * b - 4
        mms.append((b, 0, 4, base - 512 * hh))
    return mms


def _build(nc):
    I_d = nc.dram_tensor("I", [H, W], F32, kind="ExternalInput").ap()
    J_d = nc.dram_tensor("J", [H, W], F32, kind="ExternalInput").ap()
    s0_d = nc.dram_tensor("s0", [H, W], F32, kind="ExternalInput").ap()
    s1_d = nc.dram_tensor("s1", [H, W], F32, kind="ExternalInput").ap()
    bandh_d = nc.dram_tensor("bandh", [128, 272], BF16,
                             kind="ExternalInput").ap()
    bandw_d = nc.dram_tensor("bandw", [128, 256], BF16,
                             kind="ExternalInput").ap()
    part_d = nc.dram_tensor("partials", [128, 68], F32,
                            kind="ExternalOutput").ap()

    MAPS = ("si", "sj", "sij", "sii", "sjj")

    from contextlib import ExitStack
    with tile.TileContext(nc) as tc, ExitStack() as ctx:
        consts = ctx.enter_context(tc.tile_pool(name="consts", bufs=1))
        inp = ctx.enter_context(tc.tile_pool(name="inp", bufs=2))
        pmap = ctx.enter_context(tc.tile_pool(name="pmap", bufs=1))
        tmap = ctx.enter_context(tc.tile_pool(name="tmap", bufs=2))
        ctmp = ctx.enter_context(tc.tile_pool(name="ctmp", bufs=2))
        spool = ctx.enter_context(tc.tile_pool(name="spool", bufs=2))
        jpool = ctx.enter_context(tc.tile_pool(name="jpool", bufs=4))
        accp = ctx.enter_context(tc.tile_pool(name="accp", bufs=1))
        psA = ctx.enter_context(tc.tile_pool(name="psA", bufs=3, space="PSUM"))
        ps2 = ctx.enter_context(tc.tile_pool(name="ps2", bufs=1, space="PSUM"))

        bandh_t = consts.tile([128, 272], BF16)
        bandw_t = consts.tile([128, 256], BF16)
        nc.sync.dma_start(bandh_t[:], bandh_d)
        nc.sync.dma_start(bandw_t[:], bandw_d)

        # accumulators: accum_out OVERWRITES, so every accumulating
        # instruction gets its own column; host sums the groups.
        # cols 0-17: cc per (wchunk,half); 18-33: lag_w; 34-49: lag_h;
        # 52-67: s^2  (50-51 unused; layout matches host assembly)
        acc = accp.tile([128, 68], F32)
        nc.vector.memset(acc[:], 0.0)

        # ---------------- premaps: 5 bf16 maps per h-block ---------------
        pm = {}
        smooth_jobs = []
        for ch_i, s_d in enumerate((s0_d, s1_d)):
            for t in range(8):
                smooth_jobs.append((ch_i, t, s_d))

        def emit_smooth(job):
            ch_i, t, s_d = job
            st = spool.tile([128, W], F32, tag="s_in")
            eng_d = nc.sync if t % 2 == 0 else nc.scalar
            eng_d.dma_start(st[:], s_d[128 * t:128 * (t + 1), :])
            # sum s^2 (output is junk; only the accumulator matters)
            s2o = jpool.tile([128, W], F32, tag="junk")
            nc.scalar.activation(s2o[:], st[:], ACTF.Square,
                                 accum_out=acc[:, 52 + ch_i * 8 + t:
                                               53 + ch_i * 8 + t])
            # lag_w: s[w]*s[w+1]
            lw = jpool.tile([128, W], F32, tag="junk")
            nc.vector.scalar_tensor_tensor(
                out=lw[:, 0:W - 1], in0=st[:, 1:W], scalar=1.0,
                in1=st[:, 0:W - 1], op0=ALU.mult, op1=ALU.mult,
                accum_out=acc[:, 18 + ch_i * 8 + t:19 + ch_i * 8 + t])
            # lag_h: row-shifted copy loaded straight from DRAM (row t*128+1
            # onward), so s[h]*s[h+1] covers tile boundaries too
            nsh = 128 if t < 7 else 127
            sh = spool.tile([128, W], F32, tag="sh")
            eng_d2 = nc.scalar if t % 2 == 0 else nc.sync
            eng_d2.dma_start(sh[0:nsh, :],
                             s_d[128 * t + 1:128 * t + 1 + nsh, :])
            lh = jpool.tile([128, W], F32, tag="junk")
            nc.vector.scalar_tensor_tensor(
                out=lh[0:nsh, :], in0=sh[0:nsh, :], scalar=1.0,
                in1=st[0:nsh, :], op0=ALU.mult, op1=ALU.mult,
                accum_out=acc[0:nsh, 34 + ch_i * 8 + t:
                              35 + ch_i * 8 + t])

        def emit_products(b):
            I_t = inp.tile([128, W], F32, tag="I_in")
            J_t = inp.tile([128, W], F32, tag="J_in")
            nc.sync.dma_start(I_t[:], I_d[128 * b:128 * (b + 1), :])
            nc.scalar.dma_start(J_t[:], J_d[128 * b:128 * (b + 1), :])
            for name in MAPS:
                pm[(name, b)] = pmap.tile([128, W], BF16,
                                          tag=f"pm_{name}_{b}",
                                          name=f"pm_{name}_{b}")
            nc.gpsimd.tensor_copy(pm[("si", b)][:], I_t[:])
            nc.gpsimd.tensor_copy(pm[("sj", b)][:], J_t[:])
            nc.scalar.square(pm[("sii", b)][:], I_t[:])
            nc.scalar.square(pm[("sjj", b)][:], J_t[:])
            nc.vector.tensor_tensor(out=pm[("sij", b)][:], in0=I_t[:],
                                    in1=J_t[:], op=ALU.mult)

        # half 0 of the image needs only h-blocks 0-4; emit those, start
        # the hh=0 sweep, and fold blocks 5-7 + smoothness into the sweep.
        for b in range(5):
            emit_products(b)
            if smooth_jobs and b >= 3:
                emit_smooth(smooth_jobs.pop(0))

        # ------------- per (hh, w-chunk): fused H-conv+transpose, -------
        # ------------- W-conv, combine ----------------------------------
        for hh in range(2):
            for c, (olo, on, ilo, inn) in enumerate(WCHUNKS):
                if hh == 0 and c < 3:
                    emit_products(5 + c)       # blocks 5-7 ride the sweep
                if smooth_jobs:
                    emit_smooth(smooth_jobs.pop(0))
                t_tiles = {}
                for mi, name in enumerate(MAPS):
                    scaled = mi >= 2
                    bh_off = 136 if scaled else 0
                    tt = tmap.tile([128, 512], BF16, tag=f"T_{name}_{hh}")
                    t_tiles[name] = tt
                    pT = psA.tile([128, 512], F32, tag="psA")
                    mms = _fused_mm_list(hh)
                    for k, (b, jlo, jn, pcol) in enumerate(mms):
                        nc.tensor.matmul(
                            pT[0:inn, pcol:pcol + jn],
                            pm[(name, b)][:, ilo:ilo + inn],
                            bandh_t[:, bh_off + jlo:bh_off + jlo + jn],
                            start=(k == 0), stop=(k == len(mms) - 1),
                            skip_group_check=True,
                        )
                    if (mi + hh + c) % 5 < 3:
                        nc.vector.tensor_copy(tt[0:inn, :], pT[0:inn, :])
                    else:
                        nc.scalar.copy(tt[0:inn, :], pT[0:inn, :])

                bw_off = 0 if c == 0 else 128
                p2 = {}
                for name in MAPS:
                    p2[name] = ps2.tile([128, 512], F32, tag=f"p2_{name}",
                                        name=f"p2_{name}_{c}_{hh}")
                    nc.tensor.matmul(p2[name][:, :],
                                     bandw_t[0:inn, bw_off:bw_off + 128],
                                     t_tiles[name][0:inn, :],
                                     start=True, stop=True)

                n = on
                # combine; bf16 intermediates, ln in fp32
                si_sb = ctmp.tile([128, 512], BF16, tag="si_sb")
                nc.scalar.copy(si_sb[0:n, :], p2["si"][0:n, :])
                P = ctmp.tile([128, 512], BF16, tag="P")
                nc.vector.scalar_tensor_tensor(
                    out=P[0:n, :], in0=si_sb[0:n, :], scalar=1.0,
                    in1=p2["sj"][0:n, :], op0=ALU.mult, op1=ALU.mult)
                crossN = ctmp.tile([128, 512], BF16, tag="crossN")
                nc.vector.scalar_tensor_tensor(
                    out=crossN[0:n, :], in0=p2["sij"][0:n, :], scalar=1.0,
                    in1=P[0:n, :], op0=ALU.mult, op1=ALU.subtract)
                si2 = ctmp.tile([128, 512], BF16, tag="si2")
                nc.gpsimd.tensor_tensor(out=si2[0:n, :], in0=si_sb[0:n, :],
                                        in1=si_sb[0:n, :], op=ALU.mult)
                IvarN = ctmp.tile([128, 512], BF16, tag="IvarN")
                nc.vector.scalar_tensor_tensor(
                    out=IvarN[0:n, :], in0=p2["sii"][0:n, :], scalar=1.0,
                    in1=si2[0:n, :], op0=ALU.mult, op1=ALU.subtract)
                sj2 = ctmp.tile([128, 512], BF16, tag="sj2")
                nc.scalar.square(sj2[0:n, :], p2["sj"][0:n, :])
                JvarN = ctmp.tile([128, 512], BF16, tag="JvarN")
                nc.vector.scalar_tensor_tensor(
                    out=JvarN[0:n, :], in0=p2["sjj"][0:n, :], scalar=1.0,
                    in1=sj2[0:n, :], op0=ALU.mult, op1=ALU.subtract)
                denom = ctmp.tile([128, 512], BF16, tag="denom")
                nc.gpsimd.tensor_tensor(out=denom[0:n, :], in0=IvarN[0:n, :],
                                        in1=JvarN[0:n, :], op=ALU.mult)
                lnd = ctmp.tile([128, 512], F32, tag="lnd")
                nc.scalar.activation(lnd[0:n, :], denom[0:n, :], ACTF.Ln)
                recip = ctmp.tile([128, 512], BF16, tag="recip")
                nc.scalar.activation(recip[0:n, :], lnd[0:n, :], ACTF.Exp,
                                     scale=-1.0)
                cross2 = ctmp.tile([128, 512], BF16, tag="cross2")
                nc.gpsimd.tensor_tensor(out=cross2[0:n, :],
                                        in0=crossN[0:n, :],
                                        in1=crossN[0:n, :], op=ALU.mult)
                ccj = ctmp.tile([128, 512], BF16, tag="ccj")
                nc.vector.scalar_tensor_tensor(
                    out=ccj[0:n, :], in0=cross2[0:n, :], scalar=1.0,
                    in1=recip[0:n, :], op0=ALU.mult, op1=ALU.mult,
                    accum_out=acc[0:n, c * 2 + hh:c * 2 + hh + 1])

        # final partition reduction happens on the host (float64)
        nc.sync.dma_start(part_d, acc[:])

    return


def _get_nc():
    if "nc" not in _nc_cache:
        nc = bass.Bass("TRN2", target_bir_lowering=False, debug=False)
        _build(nc)
        _legalize_waits(nc)
        _nc_cache["nc"] = nc
    return _nc_cache["nc"]


def _make_in_maps(I, J, s):
    B = I.shape[0]
    consts = _make_host_consts()
    in_maps = []
    for b in range(B):
        m = {
            "I": np.ascontiguousarray(I[b, 0]),
            "J": np.ascontiguousarray(J[b, 0]),
            "s0": np.ascontiguousarray(s[b, 0]),
            "s1": np.ascontiguousarray(s[b, 1]),
        }
        m.update(consts)
        in_maps.append(m)
    return in_maps


def kernel(I, J, s, sum_filt):
    B = I.shape[0]
    assert I.shape == (B, 1, H, W) and s.shape == (B, 2, H, W)
    nc = _get_nc()
    in_maps = _make_in_maps(I, J, s)
    res = bass_utils.run_bass_kernel_spmd(nc, in_maps,
                                          core_ids=list(range(B)))
    parts = np.stack([res.results[b]["partials"] for b in range(B)])
    parts = parts.astype(np.float64).sum(axis=1)   # reduce partition dim

    # host-side final assembly (float64)
    s64 = s.astype(np.float64)
    cc_sum = float(parts[:, 0:18].sum())
    lag_w = parts[:, 18:34].sum(axis=1)
    lag_h = parts[:, 34:52].sum(axis=1)
    s2 = parts[:, 52:68].sum(axis=1)

    # edge corrections per core (both channels folded together)
    e_w = (s64[:, :, :, 0] ** 2).sum(axis=(1, 2)) + \
          (s64[:, :, :, -1] ** 2).sum(axis=(1, 2))
    e_h = (s64[:, :, 0, :] ** 2).sum(axis=(1, 2)) + \
          (s64[:, :, -1, :] ** 2).sum(axis=(1, 2))

    sum_dx2 = (2.0 * s2 - e_w - 2.0 * lag_w).sum()
    sum_dy2 = (2.0 * s2 - e_h - 2.0 * lag_h).sum()
    cnt = B * 2 * H * (W - 1)

    ncc_loss = -cc_sum / (B * H * W)
    smooth = 0.5 * (sum_dx2 / cnt + sum_dy2 / cnt) * ALPHA
    total = ncc_loss + smooth
    return np.array([total, ncc_loss, smooth], dtype=np.float32)


# revision 12
# speedup vs baseline: 2.2658x; 1.0898x over previous
"""Trainium2 Bass kernel for LocalCrossCorrelationWithSmoothnessLoss.

Full inputs in, full output out. Pure data-parallel over batch (B=8 -> 8
NeuronCores); each core computes partial sums for its image; the host
combines them into the three scalar losses.

Per-core pipeline (one 1024x1024 image pair + two flow channels):
  premaps   I,J cast to bf16 (GPSIMD); I^2,J^2 (ACT); I*J (DVE) -> 5 bf16
            maps resident in SBUF.
  stage 1   fused transpose + H-direction 9-tap box conv on the PE:
            stationary = 128x128 premap block, moving = banded H matrix
            (81-scaled for the product maps)  ->  PSUM [w, h] fp32.
            This replaces the baseline's separate transpose pass.
  T-copy    PSUM -> SBUF bf16 (DVE/ACT alternating), chunked at stride
            120 along w with the 4-wide halo baked into the chunking.
  stage 2   W-direction box conv: stationary = banded W matrix, moving =
            T chunk -> PSUM [w_out, h] fp32.
  combine   crossN = 81S_IJ - S_I*S_J, IvarN = 81S_II - S_I^2,
            JvarN = 81S_JJ - S_J^2, cc = crossN^2 * exp(-ln(IvarN*JvarN))
            read directly from PSUM, bf16 intermediates (ln in fp32),
            split across DVE/ACT/GPSIMD, accumulated per-partition.
  smooth    sum(s^2) (ACT accum), lag products (DVE STT accum; row shift
            via SBUF->SBUF DMA). Edge corrections on the host.

Output per core: 68 partial sums. Host assembles the losses in float64.
"""
import sys
import numpy as np

sys.path.insert(0, "/opt/trn_rl_repo")

import ml_dtypes
import bass_rust
import concourse.bass as bass
import concourse.tile as tile
from concourse import mybir
from concourse import bass_utils
from concourse import tile_utils

F32 = mybir.dt.float32
BF16 = mybir.dt.bfloat16
ALU = mybir.AluOpType
ACTF = mybir.ActivationFunctionType

H = 1024
W = 1024
PAD = 4
ALPHA = 0.01
STRIDE = 120
NB = 8            # h blocks of 128

# W-chunk table: (out_lo, out_n, in_lo, in_n)
WCHUNKS = []
for _c in range((W + STRIDE - 1) // STRIDE):
    _olo = STRIDE * _c
    _on = min(STRIDE, W - _olo)
    _ilo = max(0, _olo - PAD)
    _ihi = min(W, _olo + _on + PAD)
    WCHUNKS.append((_olo, _on, _ilo, _ihi - _ilo))
NWC = len(WCHUNKS)

tile_utils.max_sbuf_usage = 206 * 1024

_nc_cache = {}


def _legalize_waits(nc, max_waits=1):
    """walrus accepts only one sync-wait per instruction; split extras
    onto same-engine NoOps placed just before."""
    ctr = 0
    for f in nc.m.functions:
        for bb in f.blocks:
            insts = bb.instructions
            i = 0
            while i < len(insts):
                ins = insts[i]
                si = ins.sync_info
                if si is None:
                    i += 1
                    continue
                w = list(si.on_wait)
                if len(w) <= max_waits:
                    i += 1
                    continue
                extra, keep = w[:-max_waits], w[-max_waits:]
                nops = []
                for j in range(0, len(extra), max_waits):
                    chunk = extra[j:j + max_waits]
                    nop = mybir.InstNoOp(name=f"I-wsplit-{ctr}", ins=[], outs=[])
                    ctr += 1
                    nop.engine = ins.engine
                    nop.sync_info = bass_rust.SyncInfo(on_wait=chunk, on_update=[])
                    nops.append(nop)
                ins.sync_info = bass_rust.SyncInfo(on_wait=keep,
                                                  on_update=list(si.on_update))
                insts[i:i] = nops
                i += len(nops) + 1


def _make_host_consts():
    """bandh [128, 272] bf16 (unscaled | x81), bandw [128, 256] bf16
    (chunk-0 variant | interior variant)."""
    bh = np.zeros((128, 136), dtype=np.float32)
    h = np.arange(128)[:, None]
    j = np.arange(136)[None, :]
    bh[(h >= j - 8) & (h <= j)] = 1.0
    bandh = np.concatenate([bh, bh * 81.0], axis=1).astype(ml_dtypes.bfloat16)

    r = np.arange(128)[:, None]
    q = np.arange(128)[None, :]
    b0 = (np.abs(r - q) <= PAD).astype(np.float32)          # chunk 0
    b1 = ((r >= q) & (r <= q + 8)).astype(np.float32)       # interior
    bandw = np.concatenate([b0, b1], axis=1).astype(ml_dtypes.bfloat16)
    return {"bandh": bandh, "bandw": bandw}


def _fused_mm_list(hh):
    """MM descriptors for one psum half tile (h_out in [512*hh, 512*hh+511]).
    Returns list of (h_block, j_lo, j_n, psum_col)."""
    mms = []
    for b in range(4 * hh, 4 * hh + 4):
        base = 128 * b - 4
        lo = max(512 * hh, base)
        hi = min(512 * hh + 511, base + 135)
        mms.append((b, lo - base, hi - lo + 1, lo - 512 * hh))
    if hh == 1:
        b = 4 * hh - 1
        base = 128 * b - 4
        mms.append((b, 512 * hh - base, 4, 0))
    if hh == 0:
        b = 4
        base = 128 * b - 4
        mms.append((b, 0, 4, base - 512 * hh))
    return mms


def _build(nc):
    I_d = nc.dram_tensor("I", [H, W], F32, kind="ExternalInput").ap()
    J_d = nc.dram_tensor("J", [H, W], F32, kind="ExternalInput").ap()
    s0_d = nc.dram_tensor("s0", [H, W], F32, kind="ExternalInput").ap()
    s1_d = nc.dram_tensor("s1", [H, W], F32, kind="ExternalInput").ap()
    bandh_d = nc.dram_tensor("bandh", [128, 272], BF16,
                             kind="ExternalInput").ap()
    bandw_d = nc.dram_tensor("bandw", [128, 256], BF16,
                             kind="ExternalInput").ap()
    part_d = nc.dram_tensor("partials", [128, 68], F32,
                            kind="ExternalOutput").ap()

    MAPS = ("si", "sj", "sij", "sii", "sjj")

    from contextlib import ExitStack
    with tile.TileContext(nc) as tc, ExitStack() as ctx:
        consts = ctx.enter_context(tc.tile_pool(name="consts", bufs=1))
        inp = ctx.enter_context(tc.tile_pool(name="inp", bufs=2))
        pmap = ctx.enter_context(tc.tile_pool(name="pmap", bufs=1))
        tmap = ctx.enter_context(tc.tile_pool(name="tmap", bufs=2))
        ctmp = ctx.enter_context(tc.tile_pool(name="ctmp", bufs=2))
        spool = ctx.enter_context(tc.tile_pool(name="spool", bufs=2))
        jpool = ctx.enter_context(tc.tile_pool(name="jpool", bufs=4))
        accp = ctx.enter_context(tc.tile_pool(name="accp", bufs=1))
        psA = ctx.enter_context(tc.tile_pool(name="psA", bufs=3, space="PSUM"))
        ps2 = ctx.enter_context(tc.tile_pool(name="ps2", bufs=1, space="PSUM"))

        bandh_t = consts.tile([128, 272], BF16)
        bandw_t = consts.tile([128, 256], BF16)
        nc.sync.dma_start(bandh_t[:], bandh_d)
        nc.sync.dma_start(bandw_t[:], bandw_d)

        # accumulators: accum_out OVERWRITES, so every accumulating
        # instruction gets its own column; host sums the groups.
        # cols 0-17: cc per (wchunk,half); 18-33: lag_w; 34-49: lag_h;
        # 52-67: s^2  (50-51 unused; layout matches host assembly)
        acc = accp.tile([128, 68], F32)
        nc.vector.memset(acc[:], 0.0)

        # ---------------- premaps: 5 bf16 maps per h-block ---------------
        pm = {}
        smooth_jobs = []
        for ch_i, s_d in enumerate((s0_d, s1_d)):
            for t in range(8):
                smooth_jobs.append((ch_i, t, s_d))

        def emit_smooth(job):
            ch_i, t, s_d = job
            st = spool.tile([128, W], F32, tag="s_in")
            eng_d = nc.sync if t % 2 == 0 else nc.scalar
            eng_d.dma_start(st[:], s_d[128 * t:128 * (t + 1), :])
            # sum s^2 (output is junk; only the accumulator matters)
            s2o = jpool.tile([128, W], F32, tag="junk")
            nc.scalar.activation(s2o[:], st[:], ACTF.Square,
                                 accum_out=acc[:, 52 + ch_i * 8 + t:
                                               53 + ch_i * 8 + t])
            # lag_w: s[w]*s[w+1]
            lw = jpool.tile([128, W], F32, tag="junk")
            nc.vector.scalar_tensor_tensor(
                out=lw[:, 0:W - 1], in0=st[:, 1:W], scalar=1.0,
                in1=st[:, 0:W - 1], op0=ALU.mult, op1=ALU.mult,
                accum_out=acc[:, 18 + ch_i * 8 + t:19 + ch_i * 8 + t])
            # lag_h: row-shifted copy loaded straight from DRAM (row t*128+1
            # onward), so s[h]*s[h+1] covers tile boundaries too
            nsh = 128 if t < 7 else 127
            sh = spool.tile([128, W], F32, tag="sh")
            eng_d2 = nc.scalar if t % 2 == 0 else nc.sync
            eng_d2.dma_start(sh[0:nsh, :],
                             s_d[128 * t + 1:128 * t + 1 + nsh, :])
            lh = jpool.tile([128, W], F32, tag="junk")
            nc.vector.scalar_tensor_tensor(
                out=lh[0:nsh, :], in0=sh[0:nsh, :], scalar=1.0,
                in1=st[0:nsh, :], op0=ALU.mult, op1=ALU.mult,
                accum_out=acc[0:nsh, 34 + ch_i * 8 + t:
                              35 + ch_i * 8 + t])

        def emit_products(b):
            I_t = inp.tile([128, W], F32, tag="I_in")
            J_t = inp.tile([128, W], F32, tag="J_in")
            nc.sync.dma_start(I_t[:], I_d[128 * b:128 * (b + 1), :])
            nc.scalar.dma_start(J_t[:], J_d[128 * b:128 * (b + 1), :])
            for name in MAPS:
                pm[(name, b)] = pmap.tile([128, W], BF16,
                                          tag=f"pm_{name}_{b}",
                                          name=f"pm_{name}_{b}")
            nc.scalar.copy(pm[("si", b)][:], I_t[:])
            nc.scalar.copy(pm[("sj", b)][:], J_t[:])
            nc.scalar.square(pm[("sii", b)][:], I_t[:])
            nc.scalar.square(pm[("sjj", b)][:], J_t[:])
            nc.gpsimd.tensor_tensor(out=pm[("sij", b)][:], in0=I_t[:],
                                    in1=J_t[:], op=ALU.mult)

        # half 0 of the image needs only h-blocks 0-4; emit those, start
        # the hh=0 sweep, and fold blocks 5-7 + smoothness into the sweep.
        for b in range(5):
            emit_products(b)
            if smooth_jobs and b >= 3:
                emit_smooth(smooth_jobs.pop(0))

        # ------------- per (hh, w-chunk): fused H-conv+transpose, -------
        # ------------- W-conv, combine ----------------------------------
        for hh in range(2):
            for c, (olo, on, ilo, inn) in enumerate(WCHUNKS):
                if hh == 0 and c < 3:
                    emit_products(5 + c)       # blocks 5-7 ride the sweep
                if smooth_jobs:
                    emit_smooth(smooth_jobs.pop(0))
                t_tiles = {}
                for mi, name in enumerate(MAPS):
                    scaled = mi >= 2
                    bh_off = 136 if scaled else 0
                    tt = tmap.tile([128, 512], BF16, tag=f"T_{name}_{hh}")
                    t_tiles[name] = tt
                    pT = psA.tile([128, 512], F32, tag="psA")
                    mms = _fused_mm_list(hh)
                    for k, (b, jlo, jn, pcol) in enumerate(mms):
                        nc.tensor.matmul(
                            pT[0:inn, pcol:pcol + jn],
                            pm[(name, b)][:, ilo:ilo + inn],
                            bandh_t[:, bh_off + jlo:bh_off + jlo + jn],
                            start=(k == 0), stop=(k == len(mms) - 1),
                            skip_group_check=True,
                        )
                    if (mi + hh + c) % 5 < 3:
                        nc.vector.tensor_copy(tt[0:inn, :], pT[0:inn, :])
                    else:
                        nc.scalar.copy(tt[0:inn, :], pT[0:inn, :])

                bw_off = 0 if c == 0 else 128
                p2 = {}
                for name in MAPS:
                    p2[name] = ps2.tile([128, 512], F32, tag=f"p2_{name}",
                                        name=f"p2_{name}_{c}_{hh}")
                    nc.tensor.matmul(p2[name][:, :],
                                     bandw_t[0:inn, bw_off:bw_off + 128],
                                     t_tiles[name][0:inn, :],
                                     start=True, stop=True)

                n = on
                # combine; bf16 intermediates, ln in fp32
                si_sb = ctmp.tile([128, 512], BF16, tag="si_sb")
                nc.scalar.copy(si_sb[0:n, :], p2["si"][0:n, :])
                P = ctmp.tile([128, 512], BF16, tag="P")
                nc.vector.scalar_tensor_tensor(
                    out=P[0:n, :], in0=si_sb[0:n, :], scalar=1.0,
                    in1=p2["sj"][0:n, :], op0=ALU.mult, op1=ALU.mult)
                crossN = ctmp.tile([128, 512], BF16, tag="crossN")
                nc.vector.scalar_tensor_tensor(
                    out=crossN[0:n, :], in0=p2["sij"][0:n, :], scalar=1.0,
                    in1=P[0:n, :], op0=ALU.mult, op1=ALU.subtract)
                si2 = ctmp.tile([128, 512], BF16, tag="si2")
                nc.gpsimd.tensor_tensor(out=si2[0:n, :], in0=si_sb[0:n, :],
                                        in1=si_sb[0:n, :], op=ALU.mult)
                IvarN = ctmp.tile([128, 512], BF16, tag="IvarN")
                nc.vector.scalar_tensor_tensor(
                    out=IvarN[0:n, :], in0=p2["sii"][0:n, :], scalar=1.0,
                    in1=si2[0:n, :], op0=ALU.mult, op1=ALU.subtract)
                sj2 = ctmp.tile([128, 512], BF16, tag="sj2")
                nc.scalar.square(sj2[0:n, :], p2["sj"][0:n, :])
                JvarN = ctmp.tile([128, 512], BF16, tag="JvarN")
                nc.vector.scalar_tensor_tensor(
                    out=JvarN[0:n, :], in0=p2["sjj"][0:n, :], scalar=1.0,
                    in1=sj2[0:n, :], op0=ALU.mult, op1=ALU.subtract)
                denom = ctmp.tile([128, 512], BF16, tag="denom")
                nc.gpsimd.tensor_tensor(out=denom[0:n, :], in0=IvarN[0:n, :],
                                        in1=JvarN[0:n, :], op=ALU.mult)
                lnd = ctmp.tile([128, 512], F32, tag="lnd")
                nc.scalar.activation(lnd[0:n, :], denom[0:n, :], ACTF.Ln)
                recip = ctmp.tile([128, 512], BF16, tag="recip")
                nc.scalar.activation(recip[0:n, :], lnd[0:n, :], ACTF.Exp,
                                     scale=-1.0)
                cross2 = ctmp.tile([128, 512], BF16, tag="cross2")
                nc.gpsimd.tensor_tensor(out=cross2[0:n, :],
                                        in0=crossN[0:n, :],
                                        in1=crossN[0:n, :], op=ALU.mult)
                ccj = ctmp.tile([128, 512], BF16, tag="ccj")
                nc.vector.scalar_tensor_tensor(
                    out=ccj[0:n, :], in0=cross2[0:n, :], scalar=1.0,
                    in1=recip[0:n, :], op0=ALU.mult, op1=ALU.mult,
                    accum_out=acc[0:n, c * 2 + hh:c * 2 + hh + 1])

        # final partition reduction happens on the host (float64)
        nc.sync.dma_start(part_d, acc[:])

    return


def _get_nc():
    if "nc" not in _nc_cache:
        nc = bass.Bass("TRN2", target_bir_lowering=False, debug=False)
        _build(nc)
        _legalize_waits(nc)
        _nc_cache["nc"] = nc
    return _nc_cache["nc"]


def _make_in_maps(I, J, s):
    B = I.shape[0]
    consts = _make_host_consts()
    in_maps = []
    for b in range(B):
        m = {
            "I": np.ascontiguousarray(I[b, 0]),
            "J": np.ascontiguousarray(J[b, 0]),
            "s0": np.ascontiguousarray(s[b, 0]),
            "s1": np.ascontiguousarray(s[b, 1]),
        }
        m.update(consts)
        in_maps.append(m)
    return in_maps


def kernel(I, J, s, sum_filt):
    B = I.shape[0]
    assert I.shape == (B, 1, H, W) and s.shape == (B, 2, H, W)
    nc = _get_nc()
    in_maps = _make_in_maps(I, J, s)
    res = bass_utils.run_bass_kernel_spmd(nc, in_maps,
                                          core_ids=list(range(B)))
    parts = np.stack([res.results[b]["partials"] for b in range(B)])
    parts = parts.astype(np.float64).sum(axis=1)   # reduce partition dim

    # host-side final assembly (float64)
    s64 = s.astype(np.float64)
    cc_sum = float(parts[:, 0:18].sum())
    lag_w = parts[:, 18:34].sum(axis=1)
    lag_h = parts[:, 34:52].sum(axis=1)
    s2 = parts[:, 52:68].sum(axis=1)

    # edge corrections per core (both channels folded together)
    e_w = (s64[:, :, :, 0] ** 2).sum(axis=(1, 2)) + \
          (s64[:, :, :, -1] ** 2).sum(axis=(1, 2))
    e_h = (s64[:, :, 0, :] ** 2).sum(axis=(1, 2)) + \
          (s64[:, :, -1, :] ** 2).sum(axis=(1, 2))

    sum_dx2 = (2.0 * s2 - e_w - 2.0 * lag_w).sum()
    sum_dy2 = (2.0 * s2 - e_h - 2.0 * lag_h).sum()
    cnt = B * 2 * H * (W - 1)

    ncc_loss = -cc_sum / (B * H * W)
    smooth = 0.5 * (sum_dx2 / cnt + sum_dy2 / cnt) * ALPHA
    total = ncc_loss + smooth
    return np.array([total, ncc_loss, smooth], dtype=np.float32)


# revision 14
# speedup vs baseline: 2.2992x; 1.0148x over previous
"""Trainium2 Bass kernel for LocalCrossCorrelationWithSmoothnessLoss.

Full inputs in, full output out. Pure data-parallel over batch (B=8 -> 8
NeuronCores); each core computes partial sums for its image; the host
combines them into the three scalar losses.

Per-core pipeline (one 1024x1024 image pair + two flow channels):
  premaps   I,J cast to bf16 (GPSIMD); I^2,J^2 (ACT); I*J (DVE) -> 5 bf16
            maps resident in SBUF.
  stage 1   fused transpose + H-direction 9-tap box conv on the PE:
            stationary = 128x128 premap block, moving = banded H matrix
            (81-scaled for the product maps)  ->  PSUM [w, h] fp32.
            This replaces the baseline's separate transpose pass.
  T-copy    PSUM -> SBUF bf16 (DVE/ACT alternating), chunked at stride
            120 along w with the 4-wide halo baked into the chunking.
  stage 2   W-direction box conv: stationary = banded W matrix, moving =
            T chunk -> PSUM [w_out, h] fp32.
  combine   crossN = 81S_IJ - S_I*S_J, IvarN = 81S_II - S_I^2,
            JvarN = 81S_JJ - S_J^2, cc = crossN^2 * exp(-ln(IvarN*JvarN))
            read directly from PSUM, bf16 intermediates (ln in fp32),
            split across DVE/ACT/GPSIMD, accumulated per-partition.
  smooth    sum(s^2) (ACT accum), lag products (DVE STT accum; row shift
            via SBUF->SBUF DMA). Edge corrections on the host.

Output per core: 68 partial sums. Host assembles the losses in float64.
"""
import sys
import numpy as np

sys.path.insert(0, "/opt/trn_rl_repo")

import ml_dtypes
import bass_rust
import concourse.bass as bass
import concourse.tile as tile
from concourse import mybir
from concourse import bass_utils
from concourse import tile_utils

F32 = mybir.dt.float32
BF16 = mybir.dt.bfloat16
ALU = mybir.AluOpType
ACTF = mybir.ActivationFunctionType

H = 1024
W = 1024
PAD = 4
ALPHA = 0.01
STRIDE = 120
NB = 8            # h blocks of 128

# W-chunk table: (out_lo, out_n, in_lo, in_n)
WCHUNKS = []
for _c in range((W + STRIDE - 1) // STRIDE):
    _olo = STRIDE * _c
    _on = min(STRIDE, W - _olo)
    _ilo = max(0, _olo - PAD)
    _ihi = min(W, _olo + _on + PAD)
    WCHUNKS.append((_olo, _on, _ilo, _ihi - _ilo))
NWC = len(WCHUNKS)

tile_utils.max_sbuf_usage = 206 * 1024

_nc_cache = {}


def _legalize_waits(nc, max_waits=1):
    """walrus accepts only one sync-wait per instruction; split extras
    onto same-engine NoOps placed just before."""
    ctr = 0
    for f in nc.m.functions:
        for bb in f.blocks:
            insts = bb.instructions
            i = 0
            while i < len(insts):
                ins = insts[i]
                si = ins.sync_info
                if si is None:
                    i += 1
                    continue
                w = list(si.on_wait)
                if len(w) <= max_waits:
                    i += 1
                    continue
                extra, keep = w[:-max_waits], w[-max_waits:]
                nops = []
                for j in range(0, len(extra), max_waits):
                    chunk = extra[j:j + max_waits]
                    nop = mybir.InstNoOp(name=f"I-wsplit-{ctr}", ins=[], outs=[])
                    ctr += 1
                    nop.engine = ins.engine
                    nop.sync_info = bass_rust.SyncInfo(on_wait=chunk, on_update=[])
                    nops.append(nop)
                ins.sync_info = bass_rust.SyncInfo(on_wait=keep,
                                                  on_update=list(si.on_update))
                insts[i:i] = nops
                i += len(nops) + 1


def _make_host_consts():
    """bandh [128, 272] bf16 (unscaled | x81), bandw [128, 256] bf16
    (chunk-0 variant | interior variant)."""
    bh = np.zeros((128, 136), dtype=np.float32)
    h = np.arange(128)[:, None]
    j = np.arange(136)[None, :]
    bh[(h >= j - 8) & (h <= j)] = 1.0
    bandh = np.concatenate([bh, bh * 81.0], axis=1).astype(ml_dtypes.bfloat16)

    r = np.arange(128)[:, None]
    q = np.arange(128)[None, :]
    b0 = (np.abs(r - q) <= PAD).astype(np.float32)          # chunk 0
    b1 = ((r >= q) & (r <= q + 8)).astype(np.float32)       # interior
    bandw = np.concatenate([b0, b1], axis=1).astype(ml_dtypes.bfloat16)
    return {"bandh": bandh, "bandw": bandw}


def _fused_mm_list(hh):
    """MM descriptors for one psum half tile (h_out in [512*hh, 512*hh+511]).
    Returns list of (h_block, j_lo, j_n, psum_col)."""
    mms = []
    for b in range(4 * hh, 4 * hh + 4):
        base = 128 * b - 4
        lo = max(512 * hh, base)
        hi = min(512 * hh + 511, base + 135)
        mms.append((b, lo - base, hi - lo + 1, lo - 512 * hh))
    if hh == 1:
        b = 4 * hh - 1
        base = 128 * b - 4
        mms.append((b, 512 * hh - base, 4, 0))
    if hh == 0:
        b = 4
        base = 128 * b - 4
        mms.append((b, 0, 4, base - 512 * hh))
    return mms


def _build(nc):
    I_d = nc.dram_tensor("I", [H, W], F32, kind="ExternalInput").ap()
    J_d = nc.dram_tensor("J", [H, W], F32, kind="ExternalInput").ap()
    s0_d = nc.dram_tensor("s0", [H, W], F32, kind="ExternalInput").ap()
    s1_d = nc.dram_tensor("s1", [H, W], F32, kind="ExternalInput").ap()
    bandh_d = nc.dram_tensor("bandh", [128, 272], BF16,
                             kind="ExternalInput").ap()
    bandw_d = nc.dram_tensor("bandw", [128, 256], BF16,
                             kind="ExternalInput").ap()
    part_d = nc.dram_tensor("partials", [128, 68], F32,
                            kind="ExternalOutput").ap()

    MAPS = ("si", "sj", "sij", "sii", "sjj")

    from contextlib import ExitStack
    with tile.TileContext(nc) as tc, ExitStack() as ctx:
        consts = ctx.enter_context(tc.tile_pool(name="consts", bufs=1))
        inp = ctx.enter_context(tc.tile_pool(name="inp", bufs=2))
        pmap = ctx.enter_context(tc.tile_pool(name="pmap", bufs=1))
        tmap = ctx.enter_context(tc.tile_pool(name="tmap", bufs=2))
        ctmp = ctx.enter_context(tc.tile_pool(name="ctmp", bufs=2))
        spool = ctx.enter_context(tc.tile_pool(name="spool", bufs=2))
        jpool = ctx.enter_context(tc.tile_pool(name="jpool", bufs=4))
        accp = ctx.enter_context(tc.tile_pool(name="accp", bufs=1))
        psA = ctx.enter_context(tc.tile_pool(name="psA", bufs=3, space="PSUM"))
        ps2 = ctx.enter_context(tc.tile_pool(name="ps2", bufs=1, space="PSUM"))

        bandh_t = consts.tile([128, 272], BF16)
        bandw_t = consts.tile([128, 256], BF16)
        nc.sync.dma_start(bandh_t[:], bandh_d)
        nc.sync.dma_start(bandw_t[:], bandw_d)

        # accumulators: accum_out OVERWRITES, so every accumulating
        # instruction gets its own column; host sums the groups.
        # cols 0-17: cc per (wchunk,half); 18-33: lag_w; 34-49: lag_h;
        # 52-67: s^2  (50-51 unused; layout matches host assembly)
        acc = accp.tile([128, 68], F32)
        nc.vector.memset(acc[:], 0.0)

        # ---------------- premaps: 5 bf16 maps per h-block ---------------
        pm = {}
        smooth_jobs = []
        for ch_i, s_d in enumerate((s0_d, s1_d)):
            for t in range(8):
                smooth_jobs.append((ch_i, t, s_d))

        def emit_smooth(job):
            ch_i, t, s_d = job
            st = spool.tile([128, W], F32, tag="s_in")
            eng_d = nc.sync if t % 2 == 0 else nc.scalar
            eng_d.dma_start(st[:], s_d[128 * t:128 * (t + 1), :])
            # sum s^2 (output is junk; only the accumulator matters;
            # bf16 junk keeps the DVE STTs in the fast mode)
            s2o = jpool.tile([128, W], BF16, tag="junk")
            nc.scalar.activation(s2o[:], st[:], ACTF.Square,
                                 accum_out=acc[:, 52 + ch_i * 8 + t:
                                               53 + ch_i * 8 + t])
            # lag_w: s[w]*s[w+1]
            lw = jpool.tile([128, W], BF16, tag="junk")
            nc.vector.scalar_tensor_tensor(
                out=lw[:, 0:W - 1], in0=st[:, 1:W], scalar=1.0,
                in1=st[:, 0:W - 1], op0=ALU.mult, op1=ALU.mult,
                accum_out=acc[:, 18 + ch_i * 8 + t:19 + ch_i * 8 + t])
            # lag_h: row-shifted copy loaded straight from DRAM (row t*128+1
            # onward), so s[h]*s[h+1] covers tile boundaries too
            nsh = 128 if t < 7 else 127
            sh = spool.tile([128, W], F32, tag="sh")
            eng_d2 = nc.scalar if t % 2 == 0 else nc.sync
            eng_d2.dma_start(sh[0:nsh, :],
                             s_d[128 * t + 1:128 * t + 1 + nsh, :])
            lh = jpool.tile([128, W], BF16, tag="junk")
            nc.vector.scalar_tensor_tensor(
                out=lh[0:nsh, :], in0=sh[0:nsh, :], scalar=1.0,
                in1=st[0:nsh, :], op0=ALU.mult, op1=ALU.mult,
                accum_out=acc[0:nsh, 34 + ch_i * 8 + t:
                              35 + ch_i * 8 + t])

        def emit_products(b):
            I_t = inp.tile([128, W], F32, tag="I_in")
            J_t = inp.tile([128, W], F32, tag="J_in")
            nc.sync.dma_start(I_t[:], I_d[128 * b:128 * (b + 1), :])
            nc.scalar.dma_start(J_t[:], J_d[128 * b:128 * (b + 1), :])
            for name in MAPS:
                pm[(name, b)] = pmap.tile([128, W], BF16,
                                          tag=f"pm_{name}_{b}",
                                          name=f"pm_{name}_{b}")
            nc.scalar.copy(pm[("si", b)][:], I_t[:])
            nc.scalar.copy(pm[("sj", b)][:], J_t[:])
            nc.scalar.square(pm[("sii", b)][:], I_t[:])
            nc.scalar.square(pm[("sjj", b)][:], J_t[:])
            nc.gpsimd.tensor_tensor(out=pm[("sij", b)][:], in0=I_t[:],
                                    in1=J_t[:], op=ALU.mult)

        # half 0 of the image needs only h-blocks 0-4; emit those, start
        # the hh=0 sweep, and fold blocks 5-7 + smoothness into the sweep.
        for b in range(5):
            emit_products(b)
            if smooth_jobs and b >= 3:
                emit_smooth(smooth_jobs.pop(0))

        # ------------- per (hh, w-chunk): fused H-conv+transpose, -------
        # ------------- W-conv, combine ----------------------------------
        for hh in range(2):
            for c, (olo, on, ilo, inn) in enumerate(WCHUNKS):
                if hh == 0 and c < 3:
                    emit_products(5 + c)       # blocks 5-7 ride the sweep
                if smooth_jobs:
                    emit_smooth(smooth_jobs.pop(0))
                t_tiles = {}
                for mi, name in enumerate(MAPS):
                    scaled = mi >= 2
                    bh_off = 136 if scaled else 0
                    tt = tmap.tile([128, 512], BF16, tag=f"T_{name}_{hh}")
                    t_tiles[name] = tt
                    pT = psA.tile([128, 512], F32, tag="psA")
                    mms = _fused_mm_list(hh)
                    for k, (b, jlo, jn, pcol) in enumerate(mms):
                        nc.tensor.matmul(
                            pT[0:inn, pcol:pcol + jn],
                            pm[(name, b)][:, ilo:ilo + inn],
                            bandh_t[:, bh_off + jlo:bh_off + jlo + jn],
                            start=(k == 0), stop=(k == len(mms) - 1),
                            skip_group_check=True,
                        )
                    if (mi + hh + c) % 5 < 3:
                        nc.vector.tensor_copy(tt[0:inn, :], pT[0:inn, :])
                    else:
                        nc.scalar.copy(tt[0:inn, :], pT[0:inn, :])

                bw_off = 0 if c == 0 else 128
                p2 = {}
                for name in MAPS:
                    p2[name] = ps2.tile([128, 512], F32, tag=f"p2_{name}",
                                        name=f"p2_{name}_{c}_{hh}")
                    nc.tensor.matmul(p2[name][:, :],
                                     bandw_t[0:inn, bw_off:bw_off + 128],
                                     t_tiles[name][0:inn, :],
                                     start=True, stop=True)

                n = on
                # combine; bf16 intermediates, ln in fp32
                si_sb = ctmp.tile([128, 512], BF16, tag="si_sb")
                nc.scalar.copy(si_sb[0:n, :], p2["si"][0:n, :])
                P = ctmp.tile([128, 512], BF16, tag="P")
                nc.vector.scalar_tensor_tensor(
                    out=P[0:n, :], in0=si_sb[0:n, :], scalar=1.0,
                    in1=p2["sj"][0:n, :], op0=ALU.mult, op1=ALU.mult)
                crossN = ctmp.tile([128, 512], BF16, tag="crossN")
                nc.vector.scalar_tensor_tensor(
                    out=crossN[0:n, :], in0=p2["sij"][0:n, :], scalar=1.0,
                    in1=P[0:n, :], op0=ALU.mult, op1=ALU.subtract)
                si2 = ctmp.tile([128, 512], BF16, tag="si2")
                nc.gpsimd.tensor_tensor(out=si2[0:n, :], in0=si_sb[0:n, :],
                                        in1=si_sb[0:n, :], op=ALU.mult)
                IvarN = ctmp.tile([128, 512], BF16, tag="IvarN")
                nc.vector.scalar_tensor_tensor(
                    out=IvarN[0:n, :], in0=p2["sii"][0:n, :], scalar=1.0,
                    in1=si2[0:n, :], op0=ALU.mult, op1=ALU.subtract)
                sj2 = ctmp.tile([128, 512], BF16, tag="sj2")
                nc.scalar.square(sj2[0:n, :], p2["sj"][0:n, :])
                JvarN = ctmp.tile([128, 512], BF16, tag="JvarN")
                nc.vector.scalar_tensor_tensor(
                    out=JvarN[0:n, :], in0=p2["sjj"][0:n, :], scalar=1.0,
                    in1=sj2[0:n, :], op0=ALU.mult, op1=ALU.subtract)
                denom = ctmp.tile([128, 512], BF16, tag="denom")
                nc.gpsimd.tensor_tensor(out=denom[0:n, :], in0=IvarN[0:n, :],
                                        in1=JvarN[0:n, :], op=ALU.mult)
                lnd = ctmp.tile([128, 512], F32, tag="lnd")
                nc.scalar.activation(lnd[0:n, :], denom[0:n, :], ACTF.Ln)
                recip = ctmp.tile([128, 512], BF16, tag="recip")
                nc.scalar.activation(recip[0:n, :], lnd[0:n, :], ACTF.Exp,
                                     scale=-1.0)
                cross2 = ctmp.tile([128, 512], BF16, tag="cross2")
                nc.gpsimd.tensor_tensor(out=cross2[0:n, :],
                                        in0=crossN[0:n, :],
                                        in1=crossN[0:n, :], op=ALU.mult)
                ccj = ctmp.tile([128, 512], BF16, tag="ccj")
                nc.vector.scalar_tensor_tensor(
                    out=ccj[0:n, :], in0=cross2[0:n, :], scalar=1.0,
                    in1=recip[0:n, :], op0=ALU.mult, op1=ALU.mult,
                    accum_out=acc[0:n, c * 2 + hh:c * 2 + hh + 1])

        # final partition reduction happens on the host (float64)
        nc.sync.dma_start(part_d, acc[:])

    return


def _get_nc():
    if "nc" not in _nc_cache:
        nc = bass.Bass("TRN2", target_bir_lowering=False, debug=False)
        _build(nc)
        _legalize_waits(nc)
        _nc_cache["nc"] = nc
    return _nc_cache["nc"]


def _make_in_maps(I, J, s):
    B = I.shape[0]
    consts = _make_host_consts()
    in_maps = []
    for b in range(B):
        m = {
            "I": np.ascontiguousarray(I[b, 0]),
            "J": np.ascontiguousarray(J[b, 0]),
            "s0": np.ascontiguousarray(s[b, 0]),
            "s1": np.ascontiguousarray(s[b, 1]),
        }
        m.update(consts)
        in_maps.append(m)
    return in_maps


def kernel(I, J, s, sum_filt):
    B = I.shape[0]
    assert I.shape == (B, 1, H, W) and s.shape == (B, 2, H, W)
    nc = _get_nc()
    in_maps = _make_in_maps(I, J, s)
    res = bass_utils.run_bass_kernel_spmd(nc, in_maps,
                                          core_ids=list(range(B)))
    parts = np.stack([res.results[b]["partials"] for b in range(B)])
    parts = parts.astype(np.float64).sum(axis=1)   # reduce partition dim

    # host-side final assembly (float64)
    s64 = s.astype(np.float64)
    cc_sum = float(parts[:, 0:18].sum())
    lag_w = parts[:, 18:34].sum(axis=1)
    lag_h = parts[:, 34:52].sum(axis=1)
    s2 = parts[:, 52:68].sum(axis=1)

    # edge corrections per core (both channels folded together)
    e_w = (s64[:, :, :, 0] ** 2).sum(axis=(1, 2)) + \
          (s64[:, :, :, -1] ** 2).sum(axis=(1, 2))
    e_h = (s64[:, :, 0, :] ** 2).sum(axis=(1, 2)) + \
          (s64[:, :, -1, :] ** 2).sum(axis=(1, 2))

    sum_dx2 = (2.0 * s2 - e_w - 2.0 * lag_w).sum()
    sum_dy2 = (2.0 * s2 - e_h - 2.0 * lag_h).sum()
    cnt = B * 2 * H * (W - 1)

    ncc_loss = -cc_sum / (B * H * W)
    smooth = 0.5 * (sum_dx2 / cnt + sum_dy2 / cnt) * ALPHA
    total = ncc_loss + smooth
    return np.array([total, ncc_loss, smooth], dtype=np.float32)


# revision 16
# speedup vs baseline: 2.3679x; 1.0299x over previous
"""Trainium2 Bass kernel for LocalCrossCorrelationWithSmoothnessLoss.

Full inputs in, full output out. Pure data-parallel over batch (B=8 -> 8
NeuronCores); each core computes partial sums for its image; the host
combines them into the three scalar losses.

Per-core pipeline (one 1024x1024 image pair + two flow channels):
  premaps   I,J cast to bf16 (GPSIMD); I^2,J^2 (ACT); I*J (DVE) -> 5 bf16
            maps resident in SBUF.
  stage 1   fused transpose + H-direction 9-tap box conv on the PE:
            stationary = 128x128 premap block, moving = banded H matrix
            (81-scaled for the product maps)  ->  PSUM [w, h] fp32.
            This replaces the baseline's separate transpose pass.
  T-copy    PSUM -> SBUF bf16 (DVE/ACT alternating), chunked at stride
            120 along w with the 4-wide halo baked into the chunking.
  stage 2   W-direction box conv: stationary = banded W matrix, moving =
            T chunk -> PSUM [w_out, h] fp32.
  combine   crossN = 81S_IJ - S_I*S_J, IvarN = 81S_II - S_I^2,
            JvarN = 81S_JJ - S_J^2, cc = crossN^2 * exp(-ln(IvarN*JvarN))
            read directly from PSUM, bf16 intermediates (ln in fp32),
            split across DVE/ACT/GPSIMD, accumulated per-partition.
  smooth    sum(s^2) (ACT accum), lag products (DVE STT accum; row shift
            via SBUF->SBUF DMA). Edge corrections on the host.

Output per core: 68 partial sums. Host assembles the losses in float64.
"""
import sys
import numpy as np

sys.path.insert(0, "/opt/trn_rl_repo")

import ml_dtypes
import bass_rust
import concourse.bass as bass
import concourse.tile as tile
from concourse import mybir
from concourse import bass_utils
from concourse import tile_utils

F32 = mybir.dt.float32
BF16 = mybir.dt.bfloat16
ALU = mybir.AluOpType
ACTF = mybir.ActivationFunctionType

H = 1024
W = 1024
PAD = 4
ALPHA = 0.01
STRIDE = 120
NB = 8            # h blocks of 128

# W-chunk table: (out_lo, out_n, in_lo, in_n)
WCHUNKS = []
for _c in range((W + STRIDE - 1) // STRIDE):
    _olo = STRIDE * _c
    _on = min(STRIDE, W - _olo)
    _ilo = max(0, _olo - PAD)
    _ihi = min(W, _olo + _on + PAD)
    WCHUNKS.append((_olo, _on, _ilo, _ihi - _ilo))
NWC = len(WCHUNKS)

tile_utils.max_sbuf_usage = 206 * 1024

_nc_cache = {}


def _legalize_waits(nc, max_waits=1):
    """walrus accepts only one sync-wait per instruction; split extras
    onto same-engine NoOps placed just before."""
    ctr = 0
    for f in nc.m.functions:
        for bb in f.blocks:
            insts = bb.instructions
            i = 0
            while i < len(insts):
                ins = insts[i]
                si = ins.sync_info
                if si is None:
                    i += 1
                    continue
                w = list(si.on_wait)
                if len(w) <= max_waits:
                    i += 1
                    continue
                extra, keep = w[:-max_waits], w[-max_waits:]
                nops = []
                for j in range(0, len(extra), max_waits):
                    chunk = extra[j:j + max_waits]
                    nop = mybir.InstNoOp(name=f"I-wsplit-{ctr}", ins=[], outs=[])
                    ctr += 1
                    nop.engine = ins.engine
                    nop.sync_info = bass_rust.SyncInfo(on_wait=chunk, on_update=[])
                    nops.append(nop)
                ins.sync_info = bass_rust.SyncInfo(on_wait=keep,
                                                  on_update=list(si.on_update))
                insts[i:i] = nops
                i += len(nops) + 1


def _make_host_consts():
    """bandh [128, 272] bf16 (unscaled | x81), bandw [128, 256] bf16
    (chunk-0 variant | interior variant)."""
    bh = np.zeros((128, 136), dtype=np.float32)
    h = np.arange(128)[:, None]
    j = np.arange(136)[None, :]
    bh[(h >= j - 8) & (h <= j)] = 1.0
    bandh = np.concatenate([bh, bh * 81.0], axis=1).astype(ml_dtypes.bfloat16)

    r = np.arange(128)[:, None]
    q = np.arange(128)[None, :]
    b0 = (np.abs(r - q) <= PAD).astype(np.float32)          # chunk 0
    b1 = ((r >= q) & (r <= q + 8)).astype(np.float32)       # interior
    bandw = np.concatenate([b0, b1], axis=1).astype(ml_dtypes.bfloat16)
    return {"bandh": bandh, "bandw": bandw}


def _fused_mm_list(hh):
    """MM descriptors for one psum half tile (h_out in [512*hh, 512*hh+511]).
    Returns list of (h_block, j_lo, j_n, psum_col)."""
    mms = []
    for b in range(4 * hh, 4 * hh + 4):
        base = 128 * b - 4
        lo = max(512 * hh, base)
        hi = min(512 * hh + 511, base + 135)
        mms.append((b, lo - base, hi - lo + 1, lo - 512 * hh))
    if hh == 1:
        b = 4 * hh - 1
        base = 128 * b - 4
        mms.append((b, 512 * hh - base, 4, 0))
    if hh == 0:
        b = 4
        base = 128 * b - 4
        mms.append((b, 0, 4, base - 512 * hh))
    return mms


def _build(nc):
    I_d = nc.dram_tensor("I", [H, W], F32, kind="ExternalInput").ap()
    J_d = nc.dram_tensor("J", [H, W], F32, kind="ExternalInput").ap()
    s0_d = nc.dram_tensor("s0", [H, W], F32, kind="ExternalInput").ap()
    s1_d = nc.dram_tensor("s1", [H, W], F32, kind="ExternalInput").ap()
    bandh_d = nc.dram_tensor("bandh", [128, 272], BF16,
                             kind="ExternalInput").ap()
    bandw_d = nc.dram_tensor("bandw", [128, 256], BF16,
                             kind="ExternalInput").ap()
    part_d = nc.dram_tensor("partials", [128, 68], F32,
                            kind="ExternalOutput").ap()

    MAPS = ("si", "sj", "sij", "sii", "sjj")

    from contextlib import ExitStack
    with tile.TileContext(nc) as tc, ExitStack() as ctx:
        consts = ctx.enter_context(tc.tile_pool(name="consts", bufs=1))
        inp = ctx.enter_context(tc.tile_pool(name="inp", bufs=2))
        pmap = ctx.enter_context(tc.tile_pool(name="pmap", bufs=1))
        tmap = ctx.enter_context(tc.tile_pool(name="tmap", bufs=2))
        ctmp = ctx.enter_context(tc.tile_pool(name="ctmp", bufs=2))
        spool = ctx.enter_context(tc.tile_pool(name="spool", bufs=2))
        jpool = ctx.enter_context(tc.tile_pool(name="jpool", bufs=4))
        accp = ctx.enter_context(tc.tile_pool(name="accp", bufs=1))
        psA = ctx.enter_context(tc.tile_pool(name="psA", bufs=3, space="PSUM"))
        ps2 = ctx.enter_context(tc.tile_pool(name="ps2", bufs=1, space="PSUM"))

        bandh_t = consts.tile([128, 272], BF16)
        bandw_t = consts.tile([128, 256], BF16)
        nc.sync.dma_start(bandh_t[:], bandh_d)
        nc.sync.dma_start(bandw_t[:], bandw_d)

        # accumulators: accum_out OVERWRITES, so every accumulating
        # instruction gets its own column; host sums the groups.
        # cols 0-17: cc per (wchunk,half); 18-33: lag_w; 34-49: lag_h;
        # 52-67: s^2  (50-51 unused; layout matches host assembly)
        acc = accp.tile([128, 68], F32)
        nc.vector.memset(acc[:], 0.0)

        # ---------------- premaps: 5 bf16 maps per h-block ---------------
        pm = {}
        smooth_jobs = []
        for ch_i, s_d in enumerate((s0_d, s1_d)):
            for t in range(8):
                smooth_jobs.append((ch_i, t, s_d))

        def emit_smooth(job):
            ch_i, t, s_d = job
            st = spool.tile([128, W], F32, tag="s_in")
            eng_d = nc.sync if t % 2 == 0 else nc.scalar
            eng_d.dma_start(st[:], s_d[128 * t:128 * (t + 1), :])
            # sum s^2 (output is junk; only the accumulator matters;
            # bf16 junk keeps the DVE STTs in the fast mode)
            s2o = jpool.tile([128, W], BF16, tag="junk")
            nc.scalar.activation(s2o[:], st[:], ACTF.Square,
                                 accum_out=acc[:, 52 + ch_i * 8 + t:
                                               53 + ch_i * 8 + t])
            # lag_w: s[w]*s[w+1]
            lw = jpool.tile([128, W], BF16, tag="junk")
            nc.vector.scalar_tensor_tensor(
                out=lw[:, 0:W - 1], in0=st[:, 1:W], scalar=1.0,
                in1=st[:, 0:W - 1], op0=ALU.mult, op1=ALU.mult,
                accum_out=acc[:, 18 + ch_i * 8 + t:19 + ch_i * 8 + t])
            # lag_h: row-shifted copy loaded straight from DRAM (row t*128+1
            # onward), so s[h]*s[h+1] covers tile boundaries too
            nsh = 128 if t < 7 else 127
            sh = spool.tile([128, W], F32, tag="sh")
            eng_d2 = nc.scalar if t % 2 == 0 else nc.sync
            eng_d2.dma_start(sh[0:nsh, :],
                             s_d[128 * t + 1:128 * t + 1 + nsh, :])
            lh = jpool.tile([128, W], BF16, tag="junk")
            nc.vector.scalar_tensor_tensor(
                out=lh[0:nsh, :], in0=sh[0:nsh, :], scalar=1.0,
                in1=st[0:nsh, :], op0=ALU.mult, op1=ALU.mult,
                accum_out=acc[0:nsh, 34 + ch_i * 8 + t:
                              35 + ch_i * 8 + t])

        def emit_products(b):
            I_t = inp.tile([128, W], F32, tag="I_in")
            J_t = inp.tile([128, W], F32, tag="J_in")
            nc.sync.dma_start(I_t[:], I_d[128 * b:128 * (b + 1), :])
            nc.scalar.dma_start(J_t[:], J_d[128 * b:128 * (b + 1), :])
            for name in MAPS:
                pm[(name, b)] = pmap.tile([128, W], BF16,
                                          tag=f"pm_{name}_{b}",
                                          name=f"pm_{name}_{b}")
            # si/sj premaps: second read of I/J as casting DMA (SWDGE);
            # desc-gen rides the otherwise idle GPSIMD queue
            nc.gpsimd.dma_start(pm[("si", b)][:],
                                I_d[128 * b:128 * (b + 1), :])
            nc.gpsimd.dma_start(pm[("sj", b)][:],
                                J_d[128 * b:128 * (b + 1), :])
            nc.scalar.square(pm[("sii", b)][:], I_t[:])
            nc.scalar.square(pm[("sjj", b)][:], J_t[:])
            nc.gpsimd.tensor_tensor(out=pm[("sij", b)][:], in0=I_t[:],
                                    in1=J_t[:], op=ALU.mult)

        # half 0 of the image needs only h-blocks 0-4; emit those, start
        # the hh=0 sweep, and fold blocks 5-7 + smoothness into the sweep.
        for b in range(5):
            emit_products(b)
            if smooth_jobs and b >= 3:
                emit_smooth(smooth_jobs.pop(0))

        # ------------- per (hh, w-chunk): fused H-conv+transpose, -------
        # ------------- W-conv, combine ----------------------------------
        for hh in range(2):
            for c, (olo, on, ilo, inn) in enumerate(WCHUNKS):
                if hh == 0 and c < 3:
                    emit_products(5 + c)       # blocks 5-7 ride the sweep
                if smooth_jobs and (hh == 1 or c % 2 == 0):
                    emit_smooth(smooth_jobs.pop(0))
                t_tiles = {}
                for mi, name in enumerate(MAPS):
                    scaled = mi >= 2
                    bh_off = 136 if scaled else 0
                    tt = tmap.tile([128, 512], BF16, tag=f"T_{name}_{hh}")
                    t_tiles[name] = tt
                    pT = psA.tile([128, 512], F32, tag="psA")
                    mms = _fused_mm_list(hh)
                    for k, (b, jlo, jn, pcol) in enumerate(mms):
                        nc.tensor.matmul(
                            pT[0:inn, pcol:pcol + jn],
                            pm[(name, b)][:, ilo:ilo + inn],
                            bandh_t[:, bh_off + jlo:bh_off + jlo + jn],
                            start=(k == 0), stop=(k == len(mms) - 1),
                            skip_group_check=True,
                        )
                    if (mi + hh + c) % 5 < 3:
                        nc.vector.tensor_copy(tt[0:inn, :], pT[0:inn, :])
                    else:
                        nc.scalar.copy(tt[0:inn, :], pT[0:inn, :])

                bw_off = 0 if c == 0 else 128
                p2 = {}
                for name in MAPS:
                    p2[name] = ps2.tile([128, 512], F32, tag=f"p2_{name}",
                                        name=f"p2_{name}_{c}_{hh}")
                    nc.tensor.matmul(p2[name][:, :],
                                     bandw_t[0:inn, bw_off:bw_off + 128],
                                     t_tiles[name][0:inn, :],
                                     start=True, stop=True)

                n = on
                # combine; bf16 intermediates, ln in fp32
                si_sb = ctmp.tile([128, 512], BF16, tag="si_sb")
                nc.scalar.copy(si_sb[0:n, :], p2["si"][0:n, :])
                P = ctmp.tile([128, 512], BF16, tag="P")
                nc.vector.scalar_tensor_tensor(
                    out=P[0:n, :], in0=si_sb[0:n, :], scalar=1.0,
                    in1=p2["sj"][0:n, :], op0=ALU.mult, op1=ALU.mult)
                crossN = ctmp.tile([128, 512], BF16, tag="crossN")
                nc.vector.scalar_tensor_tensor(
                    out=crossN[0:n, :], in0=p2["sij"][0:n, :], scalar=1.0,
                    in1=P[0:n, :], op0=ALU.mult, op1=ALU.subtract)
                si2 = ctmp.tile([128, 512], BF16, tag="si2")
                nc.gpsimd.tensor_tensor(out=si2[0:n, :], in0=si_sb[0:n, :],
                                        in1=si_sb[0:n, :], op=ALU.mult)
                IvarN = ctmp.tile([128, 512], BF16, tag="IvarN")
                nc.vector.scalar_tensor_tensor(
                    out=IvarN[0:n, :], in0=p2["sii"][0:n, :], scalar=1.0,
                    in1=si2[0:n, :], op0=ALU.mult, op1=ALU.subtract)
                sj2 = ctmp.tile([128, 512], BF16, tag="sj2")
                nc.scalar.square(sj2[0:n, :], p2["sj"][0:n, :])
                JvarN = ctmp.tile([128, 512], BF16, tag="JvarN")
                nc.vector.scalar_tensor_tensor(
                    out=JvarN[0:n, :], in0=p2["sjj"][0:n, :], scalar=1.0,
                    in1=sj2[0:n, :], op0=ALU.mult, op1=ALU.subtract)
                denom = ctmp.tile([128, 512], BF16, tag="denom")
                nc.gpsimd.tensor_tensor(out=denom[0:n, :], in0=IvarN[0:n, :],
                                        in1=JvarN[0:n, :], op=ALU.mult)
                lnd = ctmp.tile([128, 512], F32, tag="lnd")
                nc.scalar.activation(lnd[0:n, :], denom[0:n, :], ACTF.Ln)
                recip = ctmp.tile([128, 512], BF16, tag="recip")
                nc.scalar.activation(recip[0:n, :], lnd[0:n, :], ACTF.Exp,
                                     scale=-1.0)
                cross2 = ctmp.tile([128, 512], BF16, tag="cross2")
                nc.gpsimd.tensor_tensor(out=cross2[0:n, :],
                                        in0=crossN[0:n, :],
                                        in1=crossN[0:n, :], op=ALU.mult)
                ccj = ctmp.tile([128, 512], BF16, tag="ccj")
                nc.vector.scalar_tensor_tensor(
                    out=ccj[0:n, :], in0=cross2[0:n, :], scalar=1.0,
                    in1=recip[0:n, :], op0=ALU.mult, op1=ALU.mult,
                    accum_out=acc[0:n, c * 2 + hh:c * 2 + hh + 1])

        # final partition reduction happens on the host (float64)
        nc.sync.dma_start(part_d, acc[:])

    return


def _get_nc():
    if "nc" not in _nc_cache:
        nc = bass.Bass("TRN2", target_bir_lowering=False, debug=False)
        _build(nc)
        _legalize_waits(nc)
        _nc_cache["nc"] = nc
    return _nc_cache["nc"]


def _make_in_maps(I, J, s):
    B = I.shape[0]
    consts = _make_host_consts()
    in_maps = []
    for b in range(B):
        m = {
            "I": np.ascontiguousarray(I[b, 0]),
            "J": np.ascontiguousarray(J[b, 0]),
            "s0": np.ascontiguousarray(s[b, 0]),
            "s1": np.ascontiguousarray(s[b, 1]),
        }
        m.update(consts)
        in_maps.append(m)
    return in_maps


def kernel(I, J, s, sum_filt):
    B = I.shape[0]
    assert I.shape == (B, 1, H, W) and s.shape == (B, 2, H, W)
    nc = _get_nc()
    in_maps = _make_in_maps(I, J, s)
    res = bass_utils.run_bass_kernel_spmd(nc, in_maps,
                                          core_ids=list(range(B)))
    parts = np.stack([res.results[b]["partials"] for b in range(B)])
    parts = parts.astype(np.float64).sum(axis=1)   # reduce partition dim

    # host-side final assembly (float64)
    s64 = s.astype(np.float64)
    cc_sum = float(parts[:, 0:18].sum())
    lag_w = parts[:, 18:34].sum(axis=1)
    lag_h = parts[:, 34:52].sum(axis=1)
    s2 = parts[:, 52:68].sum(axis=1)

    # edge corrections per core (both channels folded together)
    e_w = (s64[:, :, :, 0] ** 2).sum(axis=(1, 2)) + \
          (s64[:, :, :, -1] ** 2).sum(axis=(1, 2))
    e_h = (s64[:, :, 0, :] ** 2).sum(axis=(1, 2)) + \
          (s64[:, :, -1, :] ** 2).sum(axis=(1, 2))

    sum_dx2 = (2.0 * s2 - e_w - 2.0 * lag_w).sum()
    sum_dy2 = (2.0 * s2 - e_h - 2.0 * lag_h).sum()
    cnt = B * 2 * H * (W - 1)

    ncc_loss = -cc_sum / (B * H * W)
    smooth = 0.5 * (sum_dx2 / cnt + sum_dy2 / cnt) * ALPHA
    total = ncc_loss + smooth
    return np.array([total, ncc_loss, smooth], dtype=np.float32)


# revision 18
# speedup vs baseline: 2.4290x; 1.0258x over previous
"""Trainium2 Bass kernel for LocalCrossCorrelationWithSmoothnessLoss.

Full inputs in, full output out. Pure data-parallel over batch (B=8 -> 8
NeuronCores); each core computes partial sums for its image; the host
combines them into the three scalar losses.

Per-core pipeline (one 1024x1024 image pair + two flow channels):
  premaps   I,J cast to bf16 (GPSIMD); I^2,J^2 (ACT); I*J (DVE) -> 5 bf16
            maps resident in SBUF.
  stage 1   fused transpose + H-direction 9-tap box conv on the PE:
            stationary = 128x128 premap block, moving = banded H matrix
            (81-scaled for the product maps)  ->  PSUM [w, h] fp32.
            This replaces the baseline's separate transpose pass.
  T-copy    PSUM -> SBUF bf16 (DVE/ACT alternating), chunked at stride
            120 along w with the 4-wide halo baked into the chunking.
  stage 2   W-direction box conv: stationary = banded W matrix, moving =
            T chunk -> PSUM [w_out, h] fp32.
  combine   crossN = 81S_IJ - S_I*S_J, IvarN = 81S_II - S_I^2,
            JvarN = 81S_JJ - S_J^2, cc = crossN^2 * exp(-ln(IvarN*JvarN))
            read directly from PSUM, bf16 intermediates (ln in fp32),
            split across DVE/ACT/GPSIMD, accumulated per-partition.
  smooth    sum(s^2) (ACT accum), lag products (DVE STT accum; row shift
            via SBUF->SBUF DMA). Edge corrections on the host.

Output per core: 68 partial sums. Host assembles the losses in float64.
"""
import sys
import numpy as np

sys.path.insert(0, "/opt/trn_rl_repo")

import ml_dtypes
import bass_rust
import concourse.bass as bass
import concourse.tile as tile
from concourse import mybir
from concourse import bass_utils
from concourse import tile_utils

F32 = mybir.dt.float32
BF16 = mybir.dt.bfloat16
ALU = mybir.AluOpType
ACTF = mybir.ActivationFunctionType

H = 1024
W = 1024
PAD = 4
ALPHA = 0.01
STRIDE = 120
NB = 8            # h blocks of 128

# W-chunk table: (out_lo, out_n, in_lo, in_n)
WCHUNKS = []
for _c in range((W + STRIDE - 1) // STRIDE):
    _olo = STRIDE * _c
    _on = min(STRIDE, W - _olo)
    _ilo = max(0, _olo - PAD)
    _ihi = min(W, _olo + _on + PAD)
    WCHUNKS.append((_olo, _on, _ilo, _ihi - _ilo))
NWC = len(WCHUNKS)

tile_utils.max_sbuf_usage = 206 * 1024

_nc_cache = {}


def _legalize_waits(nc, max_waits=1):
    """walrus accepts only one sync-wait per instruction; split extras
    onto same-engine NoOps placed just before."""
    ctr = 0
    for f in nc.m.functions:
        for bb in f.blocks:
            insts = bb.instructions
            i = 0
            while i < len(insts):
                ins = insts[i]
                si = ins.sync_info
                if si is None:
                    i += 1
                    continue
                w = list(si.on_wait)
                if len(w) <= max_waits:
                    i += 1
                    continue
                extra, keep = w[:-max_waits], w[-max_waits:]
                nops = []
                for j in range(0, len(extra), max_waits):
                    chunk = extra[j:j + max_waits]
                    nop = mybir.InstNoOp(name=f"I-wsplit-{ctr}", ins=[], outs=[])
                    ctr += 1
                    nop.engine = ins.engine
                    nop.sync_info = bass_rust.SyncInfo(on_wait=chunk, on_update=[])
                    nops.append(nop)
                ins.sync_info = bass_rust.SyncInfo(on_wait=keep,
                                                  on_update=list(si.on_update))
                insts[i:i] = nops
                i += len(nops) + 1


def _make_host_consts():
    """bandh [128, 272] bf16 (unscaled | x81), bandw [128, 256] bf16
    (chunk-0 variant | interior variant)."""
    bh = np.zeros((128, 136), dtype=np.float32)
    h = np.arange(128)[:, None]
    j = np.arange(136)[None, :]
    bh[(h >= j - 8) & (h <= j)] = 1.0
    bandh = np.concatenate([bh, bh * 81.0], axis=1).astype(ml_dtypes.bfloat16)

    r = np.arange(128)[:, None]
    q = np.arange(128)[None, :]
    b0 = (np.abs(r - q) <= PAD).astype(np.float32)          # chunk 0
    b1 = ((r >= q) & (r <= q + 8)).astype(np.float32)       # interior
    bandw = np.concatenate([b0, b1], axis=1).astype(ml_dtypes.bfloat16)
    return {"bandh": bandh, "bandw": bandw}


def _fused_mm_list(hh):
    """MM descriptors for one psum half tile (h_out in [512*hh, 512*hh+511]).
    Returns list of (h_block, j_lo, j_n, psum_col)."""
    mms = []
    for b in range(4 * hh, 4 * hh + 4):
        base = 128 * b - 4
        lo = max(512 * hh, base)
        hi = min(512 * hh + 511, base + 135)
        mms.append((b, lo - base, hi - lo + 1, lo - 512 * hh))
    if hh == 1:
        b = 4 * hh - 1
        base = 128 * b - 4
        mms.append((b, 512 * hh - base, 4, 0))
    if hh == 0:
        b = 4
        base = 128 * b - 4
        mms.append((b, 0, 4, base - 512 * hh))
    return mms


def _build(nc):
    I_d = nc.dram_tensor("I", [H, W], F32, kind="ExternalInput").ap()
    J_d = nc.dram_tensor("J", [H, W], F32, kind="ExternalInput").ap()
    s0_d = nc.dram_tensor("s0", [H, W], F32, kind="ExternalInput").ap()
    s1_d = nc.dram_tensor("s1", [H, W], F32, kind="ExternalInput").ap()
    bandh_d = nc.dram_tensor("bandh", [128, 272], BF16,
                             kind="ExternalInput").ap()
    bandw_d = nc.dram_tensor("bandw", [128, 256], BF16,
                             kind="ExternalInput").ap()
    part_d = nc.dram_tensor("partials", [128, 68], F32,
                            kind="ExternalOutput").ap()

    MAPS = ("si", "sj", "sij", "sii", "sjj")

    from contextlib import ExitStack
    with tile.TileContext(nc) as tc, ExitStack() as ctx:
        consts = ctx.enter_context(tc.tile_pool(name="consts", bufs=1))
        inp = ctx.enter_context(tc.tile_pool(name="inp", bufs=2))
        pmap = ctx.enter_context(tc.tile_pool(name="pmap", bufs=1))
        tmap = ctx.enter_context(tc.tile_pool(name="tmap", bufs=2))
        ctmp = ctx.enter_context(tc.tile_pool(name="ctmp", bufs=2))
        spool = ctx.enter_context(tc.tile_pool(name="spool", bufs=2))
        jpool = ctx.enter_context(tc.tile_pool(name="jpool", bufs=4))
        accp = ctx.enter_context(tc.tile_pool(name="accp", bufs=1))
        psA = ctx.enter_context(tc.tile_pool(name="psA", bufs=3, space="PSUM"))
        ps2 = ctx.enter_context(tc.tile_pool(name="ps2", bufs=1, space="PSUM"))

        bandh_t = consts.tile([128, 272], BF16)
        bandw_t = consts.tile([128, 256], BF16)
        nc.sync.dma_start(bandh_t[:], bandh_d)
        nc.sync.dma_start(bandw_t[:], bandw_d)

        # accumulators: accum_out OVERWRITES, so every accumulating
        # instruction gets its own column; host sums the groups.
        # cols 0-17: cc per (wchunk,half); 18-33: lag_w; 34-49: lag_h;
        # 52-67: s^2  (50-51 unused; layout matches host assembly)
        acc = accp.tile([128, 68], F32)
        nc.vector.memset(acc[:], 0.0)

        # ---------------- premaps: 5 bf16 maps per h-block ---------------
        pm = {}
        smooth_jobs = []
        for ch_i, s_d in enumerate((s0_d, s1_d)):
            for t in range(8):
                smooth_jobs.append((ch_i, t, s_d))

        def emit_smooth(job):
            ch_i, t, s_d = job
            st = spool.tile([128, W], F32, tag="s_in")
            eng_d = nc.sync if t % 2 == 0 else nc.scalar
            eng_d.dma_start(st[:], s_d[128 * t:128 * (t + 1), :])
            # sum s^2 (output is junk; only the accumulator matters;
            # bf16 junk keeps the DVE STTs in the fast mode)
            s2o = jpool.tile([128, W], BF16, tag="junk")
            nc.scalar.activation(s2o[:], st[:], ACTF.Square,
                                 accum_out=acc[:, 52 + ch_i * 8 + t:
                                               53 + ch_i * 8 + t])
            # lag_w: s[w]*s[w+1]
            lw = jpool.tile([128, W], BF16, tag="junk")
            nc.vector.scalar_tensor_tensor(
                out=lw[:, 0:W - 1], in0=st[:, 1:W], scalar=1.0,
                in1=st[:, 0:W - 1], op0=ALU.mult, op1=ALU.mult,
                accum_out=acc[:, 18 + ch_i * 8 + t:19 + ch_i * 8 + t])
            # lag_h: row-shifted copy loaded straight from DRAM (row t*128+1
            # onward), so s[h]*s[h+1] covers tile boundaries too
            nsh = 128 if t < 7 else 127
            sh = spool.tile([128, W], F32, tag="sh")
            eng_d2 = nc.scalar if t % 2 == 0 else nc.sync
            eng_d2.dma_start(sh[0:nsh, :],
                             s_d[128 * t + 1:128 * t + 1 + nsh, :])
            lh = jpool.tile([128, W], BF16, tag="junk")
            nc.vector.scalar_tensor_tensor(
                out=lh[0:nsh, :], in0=sh[0:nsh, :], scalar=1.0,
                in1=st[0:nsh, :], op0=ALU.mult, op1=ALU.mult,
                accum_out=acc[0:nsh, 34 + ch_i * 8 + t:
                              35 + ch_i * 8 + t])

        def emit_products(b):
            I_t = inp.tile([128, W], F32, tag="I_in")
            J_t = inp.tile([128, W], F32, tag="J_in")
            nc.sync.dma_start(I_t[:], I_d[128 * b:128 * (b + 1), :])
            nc.scalar.dma_start(J_t[:], J_d[128 * b:128 * (b + 1), :])
            for name in MAPS:
                pm[(name, b)] = pmap.tile([128, W], BF16,
                                          tag=f"pm_{name}_{b}",
                                          name=f"pm_{name}_{b}")
            # si/sj premaps: second read of I/J as casting DMA (SWDGE);
            # desc-gen rides the otherwise idle GPSIMD queue
            nc.gpsimd.dma_start(pm[("si", b)][:],
                                I_d[128 * b:128 * (b + 1), :])
            nc.gpsimd.dma_start(pm[("sj", b)][:],
                                J_d[128 * b:128 * (b + 1), :])
            nc.scalar.square(pm[("sii", b)][:], I_t[:])
            nc.scalar.square(pm[("sjj", b)][:], J_t[:])
            # early blocks gate the first fused MMs; GPSIMD's queue is busy
            # with SWDGE desc-gen then, so keep the ramp off it
            eng_ij = nc.vector if b < 5 else nc.gpsimd
            eng_ij.tensor_tensor(out=pm[("sij", b)][:], in0=I_t[:],
                                 in1=J_t[:], op=ALU.mult)

        # half 0 of the image needs only h-blocks 0-4; emit those, start
        # the hh=0 sweep, and fold blocks 5-7 + smoothness into the sweep.
        for b in range(5):
            emit_products(b)
            if smooth_jobs and b >= 3:
                emit_smooth(smooth_jobs.pop(0))

        # ------------- per (hh, w-chunk): fused H-conv+transpose, -------
        # ------------- W-conv, combine ----------------------------------
        for hh in range(2):
            for c, (olo, on, ilo, inn) in enumerate(WCHUNKS):
                if hh == 0 and c < 3:
                    emit_products(5 + c)       # blocks 5-7 ride the sweep
                if smooth_jobs and (hh == 1 or c % 2 == 0):
                    emit_smooth(smooth_jobs.pop(0))
                t_tiles = {}
                for mi, name in enumerate(MAPS):
                    scaled = mi >= 2
                    bh_off = 136 if scaled else 0
                    tt = tmap.tile([128, 512], BF16, tag=f"T_{name}_{hh}")
                    t_tiles[name] = tt
                    pT = psA.tile([128, 512], F32, tag="psA")
                    mms = _fused_mm_list(hh)
                    for k, (b, jlo, jn, pcol) in enumerate(mms):
                        nc.tensor.matmul(
                            pT[0:inn, pcol:pcol + jn],
                            pm[(name, b)][:, ilo:ilo + inn],
                            bandh_t[:, bh_off + jlo:bh_off + jlo + jn],
                            start=(k == 0), stop=(k == len(mms) - 1),
                            skip_group_check=True,
                        )
                    if (mi + hh + c) % 5 < 2:
                        nc.vector.tensor_copy(tt[0:inn, :], pT[0:inn, :])
                    else:
                        nc.scalar.copy(tt[0:inn, :], pT[0:inn, :])

                bw_off = 0 if c == 0 else 128
                p2 = {}
                for name in MAPS:
                    p2[name] = ps2.tile([128, 512], F32, tag=f"p2_{name}",
                                        name=f"p2_{name}_{c}_{hh}")
                    nc.tensor.matmul(p2[name][:, :],
                                     bandw_t[0:inn, bw_off:bw_off + 128],
                                     t_tiles[name][0:inn, :],
                                     start=True, stop=True)

                n = on
                # combine; bf16 intermediates, ln in fp32
                si_sb = ctmp.tile([128, 512], BF16, tag="si_sb")
                nc.scalar.copy(si_sb[0:n, :], p2["si"][0:n, :])
                P = ctmp.tile([128, 512], BF16, tag="P")
                nc.vector.scalar_tensor_tensor(
                    out=P[0:n, :], in0=si_sb[0:n, :], scalar=1.0,
                    in1=p2["sj"][0:n, :], op0=ALU.mult, op1=ALU.mult)
                crossN = ctmp.tile([128, 512], BF16, tag="crossN")
                nc.vector.scalar_tensor_tensor(
                    out=crossN[0:n, :], in0=p2["sij"][0:n, :], scalar=1.0,
                    in1=P[0:n, :], op0=ALU.mult, op1=ALU.subtract)
                si2 = ctmp.tile([128, 512], BF16, tag="si2")
                nc.gpsimd.tensor_tensor(out=si2[0:n, :], in0=si_sb[0:n, :],
                                        in1=si_sb[0:n, :], op=ALU.mult)
                IvarN = ctmp.tile([128, 512], BF16, tag="IvarN")
                nc.vector.scalar_tensor_tensor(
                    out=IvarN[0:n, :], in0=p2["sii"][0:n, :], scalar=1.0,
                    in1=si2[0:n, :], op0=ALU.mult, op1=ALU.subtract)
                sj2 = ctmp.tile([128, 512], BF16, tag="sj2")
                nc.scalar.square(sj2[0:n, :], p2["sj"][0:n, :])
                JvarN = ctmp.tile([128, 512], BF16, tag="JvarN")
                nc.vector.scalar_tensor_tensor(
                    out=JvarN[0:n, :], in0=p2["sjj"][0:n, :], scalar=1.0,
                    in1=sj2[0:n, :], op0=ALU.mult, op1=ALU.subtract)
                denom = ctmp.tile([128, 512], BF16, tag="denom")
                nc.gpsimd.tensor_tensor(out=denom[0:n, :], in0=IvarN[0:n, :],
                                        in1=JvarN[0:n, :], op=ALU.mult)
                lnd = ctmp.tile([128, 512], F32, tag="lnd")
                nc.scalar.activation(lnd[0:n, :], denom[0:n, :], ACTF.Ln)
                recip = ctmp.tile([128, 512], BF16, tag="recip")
                nc.scalar.activation(recip[0:n, :], lnd[0:n, :], ACTF.Exp,
                                     scale=-1.0)
                cross2 = ctmp.tile([128, 512], BF16, tag="cross2")
                nc.gpsimd.tensor_tensor(out=cross2[0:n, :],
                                        in0=crossN[0:n, :],
                                        in1=crossN[0:n, :], op=ALU.mult)
                ccj = ctmp.tile([128, 512], BF16, tag="ccj")
                nc.vector.scalar_tensor_tensor(
                    out=ccj[0:n, :], in0=cross2[0:n, :], scalar=1.0,
                    in1=recip[0:n, :], op0=ALU.mult, op1=ALU.mult,
                    accum_out=acc[0:n, c * 2 + hh:c * 2 + hh + 1])

        # final partition reduction happens on the host (float64)
        nc.sync.dma_start(part_d, acc[:])

    return


def _get_nc():
    if "nc" not in _nc_cache:
        nc = bass.Bass("TRN2", target_bir_lowering=False, debug=False)
        _build(nc)
        _legalize_waits(nc)
        _nc_cache["nc"] = nc
    return _nc_cache["nc"]


def _make_in_maps(I, J, s):
    B = I.shape[0]
    consts = _make_host_consts()
    in_maps = []
    for b in range(B):
        m = {
            "I": np.ascontiguousarray(I[b, 0]),
            "J": np.ascontiguousarray(J[b, 0]),
            "s0": np.ascontiguousarray(s[b, 0]),
            "s1": np.ascontiguousarray(s[b, 1]),
        }
        m.update(consts)
        in_maps.append(m)
    return in_maps


def kernel(I, J, s, sum_filt):
    B = I.shape[0]
    assert I.shape == (B, 1, H, W) and s.shape == (B, 2, H, W)
    nc = _get_nc()
    in_maps = _make_in_maps(I, J, s)
    res = bass_utils.run_bass_kernel_spmd(nc, in_maps,
                                          core_ids=list(range(B)))
    parts = np.stack([res.results[b]["partials"] for b in range(B)])
    parts = parts.astype(np.float64).sum(axis=1)   # reduce partition dim

    # host-side final assembly (float64)
    s64 = s.astype(np.float64)
    cc_sum = float(parts[:, 0:18].sum())
    lag_w = parts[:, 18:34].sum(axis=1)
    lag_h = parts[:, 34:52].sum(axis=1)
    s2 = parts[:, 52:68].sum(axis=1)

    # edge corrections per core (both channels folded together)
    e_w = (s64[:, :, :, 0] ** 2).sum(axis=(1, 2)) + \
          (s64[:, :, :, -1] ** 2).sum(axis=(1, 2))
    e_h = (s64[:, :, 0, :] ** 2).sum(axis=(1, 2)) + \
          (s64[:, :, -1, :] ** 2).sum(axis=(1, 2))

    sum_dx2 = (2.0 * s2 - e_w - 2.0 * lag_w).sum()
    sum_dy2 = (2.0 * s2 - e_h - 2.0 * lag_h).sum()
    cnt = B * 2 * H * (W - 1)

    ncc_loss = -cc_sum / (B * H * W)
    smooth = 0.5 * (sum_dx2 / cnt + sum_dy2 / cnt) * ALPHA
    total = ncc_loss + smooth
    return np.array([total, ncc_loss, smooth], dtype=np.float32)


# revision 21
# speedup vs baseline: 2.4487x; 1.0081x over previous
"""Trainium2 Bass kernel for LocalCrossCorrelationWithSmoothnessLoss.

Full inputs in, full output out. Pure data-parallel over batch (B=8 -> 8
NeuronCores); each core computes partial sums for its image; the host
combines them into the three scalar losses.

Per-core pipeline (one 1024x1024 image pair + two flow channels):
  premaps   I,J cast to bf16 (GPSIMD); I^2,J^2 (ACT); I*J (DVE) -> 5 bf16
            maps resident in SBUF.
  stage 1   fused transpose + H-direction 9-tap box conv on the PE:
            stationary = 128x128 premap block, moving = banded H matrix
            (81-scaled for the product maps)  ->  PSUM [w, h] fp32.
            This replaces the baseline's separate transpose pass.
  T-copy    PSUM -> SBUF bf16 (DVE/ACT alternating), chunked at stride
            120 along w with the 4-wide halo baked into the chunking.
  stage 2   W-direction box conv: stationary = banded W matrix, moving =
            T chunk -> PSUM [w_out, h] fp32.
  combine   crossN = 81S_IJ - S_I*S_J, IvarN = 81S_II - S_I^2,
            JvarN = 81S_JJ - S_J^2, cc = crossN^2 * exp(-ln(IvarN*JvarN))
            read directly from PSUM, bf16 intermediates (ln in fp32),
            split across DVE/ACT/GPSIMD, accumulated per-partition.
  smooth    sum(s^2) (ACT accum), lag products (DVE STT accum; row shift
            via SBUF->SBUF DMA). Edge corrections on the host.

Output per core: 68 partial sums. Host assembles the losses in float64.
"""
import sys
import numpy as np

sys.path.insert(0, "/opt/trn_rl_repo")

import ml_dtypes
import bass_rust
import concourse.bass as bass
import concourse.tile as tile
from concourse import mybir
from concourse import bass_utils
from concourse import tile_utils

F32 = mybir.dt.float32
BF16 = mybir.dt.bfloat16
ALU = mybir.AluOpType
ACTF = mybir.ActivationFunctionType

H = 1024
W = 1024
PAD = 4
ALPHA = 0.01
STRIDE = 120
NB = 8            # h blocks of 128

# W-chunk table: (out_lo, out_n, in_lo, in_n)
WCHUNKS = []
for _c in range((W + STRIDE - 1) // STRIDE):
    _olo = STRIDE * _c
    _on = min(STRIDE, W - _olo)
    _ilo = max(0, _olo - PAD)
    _ihi = min(W, _olo + _on + PAD)
    WCHUNKS.append((_olo, _on, _ilo, _ihi - _ilo))
NWC = len(WCHUNKS)

tile_utils.max_sbuf_usage = 206 * 1024

_nc_cache = {}


def _legalize_waits(nc, max_waits=1):
    """walrus accepts only one sync-wait per instruction; split extras
    onto same-engine NoOps placed just before."""
    ctr = 0
    for f in nc.m.functions:
        for bb in f.blocks:
            insts = bb.instructions
            i = 0
            while i < len(insts):
                ins = insts[i]
                si = ins.sync_info
                if si is None:
                    i += 1
                    continue
                w = list(si.on_wait)
                if len(w) <= max_waits:
                    i += 1
                    continue
                extra, keep = w[:-max_waits], w[-max_waits:]
                nops = []
                for j in range(0, len(extra), max_waits):
                    chunk = extra[j:j + max_waits]
                    nop = mybir.InstNoOp(name=f"I-wsplit-{ctr}", ins=[], outs=[])
                    ctr += 1
                    nop.engine = ins.engine
                    nop.sync_info = bass_rust.SyncInfo(on_wait=chunk, on_update=[])
                    nops.append(nop)
                ins.sync_info = bass_rust.SyncInfo(on_wait=keep,
                                                  on_update=list(si.on_update))
                insts[i:i] = nops
                i += len(nops) + 1


def _make_host_consts():
    """bandh [128, 272] bf16 (unscaled | x81), bandw [128, 256] bf16
    (chunk-0 variant | interior variant)."""
    bh = np.zeros((128, 136), dtype=np.float32)
    h = np.arange(128)[:, None]
    j = np.arange(136)[None, :]
    bh[(h >= j - 8) & (h <= j)] = 1.0
    bandh = np.concatenate([bh, bh * 81.0], axis=1).astype(ml_dtypes.bfloat16)

    r = np.arange(128)[:, None]
    q = np.arange(128)[None, :]
    b0 = (np.abs(r - q) <= PAD).astype(np.float32)          # chunk 0
    b1 = ((r >= q) & (r <= q + 8)).astype(np.float32)       # interior
    bandw = np.concatenate([b0, b1], axis=1).astype(ml_dtypes.bfloat16)
    return {"bandh": bandh, "bandw": bandw}


def _fused_mm_list(hh):
    """MM descriptors for one psum half tile (h_out in [512*hh, 512*hh+511]).
    Returns list of (h_block, j_lo, j_n, psum_col)."""
    mms = []
    for b in range(4 * hh, 4 * hh + 4):
        base = 128 * b - 4
        lo = max(512 * hh, base)
        hi = min(512 * hh + 511, base + 135)
        mms.append((b, lo - base, hi - lo + 1, lo - 512 * hh))
    if hh == 1:
        b = 4 * hh - 1
        base = 128 * b - 4
        mms.append((b, 512 * hh - base, 4, 0))
    if hh == 0:
        b = 4
        base = 128 * b - 4
        mms.append((b, 0, 4, base - 512 * hh))
    return mms


def _build(nc):
    I_d = nc.dram_tensor("I", [H, W], F32, kind="ExternalInput").ap()
    J_d = nc.dram_tensor("J", [H, W], F32, kind="ExternalInput").ap()
    s0_d = nc.dram_tensor("s0", [H, W], F32, kind="ExternalInput").ap()
    s1_d = nc.dram_tensor("s1", [H, W], F32, kind="ExternalInput").ap()
    bandh_d = nc.dram_tensor("bandh", [128, 272], BF16,
                             kind="ExternalInput").ap()
    bandw_d = nc.dram_tensor("bandw", [128, 256], BF16,
                             kind="ExternalInput").ap()
    part_d = nc.dram_tensor("partials", [128, 68], F32,
                            kind="ExternalOutput").ap()

    MAPS = ("si", "sj", "sij", "sii", "sjj")

    from contextlib import ExitStack
    with tile.TileContext(nc) as tc, ExitStack() as ctx:
        consts = ctx.enter_context(tc.tile_pool(name="consts", bufs=1))
        inp = ctx.enter_context(tc.tile_pool(name="inp", bufs=2))
        pmap = ctx.enter_context(tc.tile_pool(name="pmap", bufs=1))
        tmap = ctx.enter_context(tc.tile_pool(name="tmap", bufs=2))
        ctmp = ctx.enter_context(tc.tile_pool(name="ctmp", bufs=2))
        spool = ctx.enter_context(tc.tile_pool(name="spool", bufs=2))
        jpool = ctx.enter_context(tc.tile_pool(name="jpool", bufs=4))
        accp = ctx.enter_context(tc.tile_pool(name="accp", bufs=1))
        psA = ctx.enter_context(tc.tile_pool(name="psA", bufs=3, space="PSUM"))
        ps2 = ctx.enter_context(tc.tile_pool(name="ps2", bufs=1, space="PSUM"))

        bandh_t = consts.tile([128, 272], BF16)
        bandw_t = consts.tile([128, 256], BF16)
        nc.sync.dma_start(bandh_t[:], bandh_d)
        nc.sync.dma_start(bandw_t[:], bandw_d)

        # accumulators: accum_out OVERWRITES, so every accumulating
        # instruction gets its own column; host sums the groups.
        # cols 0-17: cc per (wchunk,half); 18-33: lag_w; 34-49: lag_h;
        # 52-67: s^2  (50-51 unused; layout matches host assembly)
        acc = accp.tile([128, 68], F32)
        nc.vector.memset(acc[:], 0.0)

        # ---------------- premaps: 5 bf16 maps per h-block ---------------
        pm = {}
        smooth_jobs = []
        for ch_i, s_d in enumerate((s0_d, s1_d)):
            for t in range(8):
                smooth_jobs.append((ch_i, t, s_d))

        def emit_smooth(job):
            ch_i, t, s_d = job
            st = spool.tile([128, W], F32, tag="s_in")
            eng_d = nc.sync if t % 2 == 0 else nc.scalar
            eng_d.dma_start(st[:], s_d[128 * t:128 * (t + 1), :])
            # sum s^2 (output is junk; only the accumulator matters;
            # bf16 junk keeps the DVE STTs in the fast mode)
            s2o = jpool.tile([128, W], BF16, tag="junk")
            nc.scalar.activation(s2o[:], st[:], ACTF.Square,
                                 accum_out=acc[:, 52 + ch_i * 8 + t:
                                               53 + ch_i * 8 + t])
            # lag_w: s[w]*s[w+1]
            lw = jpool.tile([128, W], BF16, tag="junk")
            nc.vector.scalar_tensor_tensor(
                out=lw[:, 0:W - 1], in0=st[:, 1:W], scalar=1.0,
                in1=st[:, 0:W - 1], op0=ALU.mult, op1=ALU.mult,
                accum_out=acc[:, 18 + ch_i * 8 + t:19 + ch_i * 8 + t])
            # lag_h: row-shifted copy loaded straight from DRAM (row t*128+1
            # onward), so s[h]*s[h+1] covers tile boundaries too
            nsh = 128 if t < 7 else 127
            sh = spool.tile([128, W], F32, tag="sh")
            eng_d2 = nc.scalar if t % 2 == 0 else nc.sync
            eng_d2.dma_start(sh[0:nsh, :],
                             s_d[128 * t + 1:128 * t + 1 + nsh, :])
            lh = jpool.tile([128, W], BF16, tag="junk")
            nc.vector.scalar_tensor_tensor(
                out=lh[0:nsh, :], in0=sh[0:nsh, :], scalar=1.0,
                in1=st[0:nsh, :], op0=ALU.mult, op1=ALU.mult,
                accum_out=acc[0:nsh, 34 + ch_i * 8 + t:
                              35 + ch_i * 8 + t])

        def emit_products(b):
            for name in MAPS:
                pm[(name, b)] = pmap.tile([128, W], BF16,
                                          tag=f"pm_{name}_{b}",
                                          name=f"pm_{name}_{b}")
            # I/J arrive as bf16 casting DMAs (SWDGE); all products derive
            # from the bf16 copies, so no fp32 staging loads at all
            nc.gpsimd.dma_start(pm[("si", b)][:],
                                I_d[128 * b:128 * (b + 1), :])
            nc.gpsimd.dma_start(pm[("sj", b)][:],
                                J_d[128 * b:128 * (b + 1), :])
            nc.scalar.square(pm[("sii", b)][:], pm[("si", b)][:])
            nc.scalar.square(pm[("sjj", b)][:], pm[("sj", b)][:])
            nc.vector.tensor_tensor(out=pm[("sij", b)][:],
                                    in0=pm[("si", b)][:],
                                    in1=pm[("sj", b)][:], op=ALU.mult)

        # half 0 of the image needs only h-blocks 0-4; emit those, start
        # the hh=0 sweep, and fold blocks 5-7 + smoothness into the sweep.
        for b in range(5):
            emit_products(b)

        # ------------- per (hh, w-chunk): fused H-conv+transpose, -------
        # ------------- W-conv, combine ----------------------------------
        for hh in range(2):
            for c, (olo, on, ilo, inn) in enumerate(WCHUNKS):
                if hh == 0 and c < 3:
                    emit_products(5 + c)       # blocks 5-7 ride the sweep
                if smooth_jobs:
                    emit_smooth(smooth_jobs.pop(0))
                t_tiles = {}
                for mi, name in enumerate(MAPS):
                    scaled = mi >= 2
                    bh_off = 136 if scaled else 0
                    tt = tmap.tile([128, 512], BF16, tag=f"T_{name}_{hh}")
                    t_tiles[name] = tt
                    pT = psA.tile([128, 512], F32, tag="psA")
                    mms = _fused_mm_list(hh)
                    for k, (b, jlo, jn, pcol) in enumerate(mms):
                        nc.tensor.matmul(
                            pT[0:inn, pcol:pcol + jn],
                            pm[(name, b)][:, ilo:ilo + inn],
                            bandh_t[:, bh_off + jlo:bh_off + jlo + jn],
                            start=(k == 0), stop=(k == len(mms) - 1),
                            skip_group_check=True,
                        )
                    if (mi + hh + c) % 5 < 2:
                        nc.vector.tensor_copy(tt[0:inn, :], pT[0:inn, :])
                    else:
                        nc.scalar.copy(tt[0:inn, :], pT[0:inn, :])

                bw_off = 0 if c == 0 else 128
                p2 = {}
                for name in MAPS:
                    p2[name] = ps2.tile([128, 512], F32, tag=f"p2_{name}",
                                        name=f"p2_{name}_{c}_{hh}")
                    nc.tensor.matmul(p2[name][:, :],
                                     bandw_t[0:inn, bw_off:bw_off + 128],
                                     t_tiles[name][0:inn, :],
                                     start=True, stop=True)

                n = on
                # combine; bf16 intermediates, ln in fp32
                si_sb = ctmp.tile([128, 512], BF16, tag="si_sb")
                nc.scalar.copy(si_sb[0:n, :], p2["si"][0:n, :])
                P = ctmp.tile([128, 512], BF16, tag="P")
                nc.vector.scalar_tensor_tensor(
                    out=P[0:n, :], in0=si_sb[0:n, :], scalar=1.0,
                    in1=p2["sj"][0:n, :], op0=ALU.mult, op1=ALU.mult)
                crossN = ctmp.tile([128, 512], BF16, tag="crossN")
                nc.vector.scalar_tensor_tensor(
                    out=crossN[0:n, :], in0=p2["sij"][0:n, :], scalar=1.0,
                    in1=P[0:n, :], op0=ALU.mult, op1=ALU.subtract)
                si2 = ctmp.tile([128, 512], BF16, tag="si2")
                nc.gpsimd.tensor_tensor(out=si2[0:n, :], in0=si_sb[0:n, :],
                                        in1=si_sb[0:n, :], op=ALU.mult)
                IvarN = ctmp.tile([128, 512], BF16, tag="IvarN")
                nc.vector.scalar_tensor_tensor(
                    out=IvarN[0:n, :], in0=p2["sii"][0:n, :], scalar=1.0,
                    in1=si2[0:n, :], op0=ALU.mult, op1=ALU.subtract)
                sj2 = ctmp.tile([128, 512], BF16, tag="sj2")
                nc.scalar.square(sj2[0:n, :], p2["sj"][0:n, :])
                JvarN = ctmp.tile([128, 512], BF16, tag="JvarN")
                nc.vector.scalar_tensor_tensor(
                    out=JvarN[0:n, :], in0=p2["sjj"][0:n, :], scalar=1.0,
                    in1=sj2[0:n, :], op0=ALU.mult, op1=ALU.subtract)
                denom = ctmp.tile([128, 512], BF16, tag="denom")
                nc.gpsimd.tensor_tensor(out=denom[0:n, :], in0=IvarN[0:n, :],
                                        in1=JvarN[0:n, :], op=ALU.mult)
                lnd = ctmp.tile([128, 512], F32, tag="lnd")
                nc.scalar.activation(lnd[0:n, :], denom[0:n, :], ACTF.Ln)
                recip = ctmp.tile([128, 512], BF16, tag="recip")
                nc.scalar.activation(recip[0:n, :], lnd[0:n, :], ACTF.Exp,
                                     scale=-1.0)
                cross2 = ctmp.tile([128, 512], BF16, tag="cross2")
                nc.gpsimd.tensor_tensor(out=cross2[0:n, :],
                                        in0=crossN[0:n, :],
                                        in1=crossN[0:n, :], op=ALU.mult)
                ccj = ctmp.tile([128, 512], BF16, tag="ccj")
                nc.vector.scalar_tensor_tensor(
                    out=ccj[0:n, :], in0=cross2[0:n, :], scalar=1.0,
                    in1=recip[0:n, :], op0=ALU.mult, op1=ALU.mult,
                    accum_out=acc[0:n, c * 2 + hh:c * 2 + hh + 1])

        # final partition reduction happens on the host (float64)
        nc.sync.dma_start(part_d, acc[:])

    return


def _get_nc():
    if "nc" not in _nc_cache:
        nc = bass.Bass("TRN2", target_bir_lowering=False, debug=False)
        _build(nc)
        _legalize_waits(nc)
        _nc_cache["nc"] = nc
    return _nc_cache["nc"]


def _make_in_maps(I, J, s):
    B = I.shape[0]
    consts = _make_host_consts()
    in_maps = []
    for b in range(B):
        m = {
            "I": np.ascontiguousarray(I[b, 0]),
            "J": np.ascontiguousarray(J[b, 0]),
            "s0": np.ascontiguousarray(s[b, 0]),
            "s1": np.ascontiguousarray(s[b, 1]),
        }
        m.update(consts)
        in_maps.append(m)
    return in_maps


def kernel(I, J, s, sum_filt):
    B = I.shape[0]
    assert I.shape == (B, 1, H, W) and s.shape == (B, 2, H, W)
    nc = _get_nc()
    in_maps = _make_in_maps(I, J, s)
    res = bass_utils.run_bass_kernel_spmd(nc, in_maps,
                                          core_ids=list(range(B)))
    parts = np.stack([res.results[b]["partials"] for b in range(B)])
    parts = parts.astype(np.float64).sum(axis=1)   # reduce partition dim

    # host-side final assembly (float64)
    s64 = s.astype(np.float64)
    cc_sum = float(parts[:, 0:18].sum())
    lag_w = parts[:, 18:34].sum(axis=1)
    lag_h = parts[:, 34:52].sum(axis=1)
    s2 = parts[:, 52:68].sum(axis=1)

    # edge corrections per core (both channels folded together)
    e_w = (s64[:, :, :, 0] ** 2).sum(axis=(1, 2)) + \
          (s64[:, :, :, -1] ** 2).sum(axis=(1, 2))
    e_h = (s64[:, :, 0, :] ** 2).sum(axis=(1, 2)) + \
          (s64[:, :, -1, :] ** 2).sum(axis=(1, 2))

    sum_dx2 = (2.0 * s2 - e_w - 2.0 * lag_w).sum()
    sum_dy2 = (2.0 * s2 - e_h - 2.0 * lag_h).sum()
    cnt = B * 2 * H * (W - 1)

    ncc_loss = -cc_sum / (B * H * W)
    smooth = 0.5 * (sum_dx2 / cnt + sum_dy2 / cnt) * ALPHA
    total = ncc_loss + smooth
    return np.array([total, ncc_loss, smooth], dtype=np.float32)
